# revision 12
# baseline (speedup 1.0000x reference)
"""Causal self-attention (B=1, T=4096, C=1024, H=16) on 8 Trainium2 cores.

Sharding: tensor-parallel over heads. Each core owns 2 heads (128 channels):
  - computes q/k/v projections for its channels against the full sequence
    (x passed transposed as [C, T] so the contraction dim lands on partitions),
  - runs causal attention for its 2 heads in S^T = [k, q] layout
    (exp on ScalarE with fused 1/8 scale; an appended ones-column in v yields
    softmax denominators as an extra output row of the same matmul),
  - multiplies its y slice against its 128 rows of wp -> a full [T, C] partial.
Host sums the 8 partials and adds the output bias. All matmuls run as
float32r (~1.5e-4 rel err, full PE speed at free-dim >= 256).
"""

import numpy as np

import concourse.bass as bass
import concourse.mybir as mybir
import concourse.tile as tile
from concourse import bacc
from concourse.bass_utils import run_bass_kernel_spmd

P = 128
N_EMBD = 1024
N_HEAD = 16
T_FULL = 4096
N_CORES = 8
D = N_EMBD // N_HEAD          # 64
C_LOCAL = 2 * D               # 128 channels per core (2 heads)
F32 = mybir.dt.float32
F32R = mybir.dt.float32r
ADD = mybir.AluOpType.add
MULT = mybir.AluOpType.mult
EXP = mybir.ActivationFunctionType.Exp

QT = 512   # attention q-tile (matmul moving free dim)
TS = 512   # projection t-chunk


def build(T=T_FULL):
    n_ts = T // TS
    n_qt = T // QT
    n_kb = T // P
    n_ct = N_EMBD // P
    kb_per_qt = QT // P

    nc = bacc.Bacc("TRN2", target_bir_lowering=False, debug=False,
                   num_devices=N_CORES)
    xT = nc.dram_tensor("xT", [N_EMBD, T], F32, kind="ExternalInput")
    w3T = nc.dram_tensor("w3T", [N_EMBD, 3 * C_LOCAL], F32, kind="ExternalInput")
    b3 = nc.dram_tensor("b3", [C_LOCAL, 3], F32, kind="ExternalInput")
    wpT = nc.dram_tensor("wpT", [C_LOCAL, N_EMBD], F32, kind="ExternalInput")
    tri = nc.dram_tensor("tri", [P, P], F32, kind="ExternalInput")
    eye = nc.dram_tensor("eye", [P, P], F32, kind="ExternalInput")
    ones = nc.dram_tensor("ones", [P, P], F32, kind="ExternalInput")
    partial = nc.dram_tensor("partial", [T, N_EMBD], F32, kind="ExternalOutput")

    xT_r = xT.ap().bitcast(F32R)

    with tile.TileContext(nc) as tc:
        with (
            tc.tile_pool(name="const", bufs=1) as cpool,
            tc.tile_pool(name="psmall", bufs=2) as psmall,
            tc.tile_pool(name="pt", bufs=3) as ptp,
            tc.tile_pool(name="osb", bufs=2) as osbp,
        ):
            # ---- constants ----
            w3_sb = cpool.tile([P, n_ct, 3 * C_LOCAL], F32R, tag="w3")
            nc.sync.dma_start(
                w3_sb[:],
                w3T.ap().bitcast(F32R).rearrange("(co p) o -> p co o", p=P))
            wp_sb = cpool.tile([P, N_EMBD], F32R, tag="wp")
            nc.sync.dma_start(wp_sb[:], wpT.ap().bitcast(F32R))
            b3_sb = cpool.tile([P, 3], F32, tag="b3")
            nc.sync.dma_start(b3_sb[:], b3.ap())
            tri_sb = cpool.tile([P, P], F32R, tag="tri")
            nc.sync.dma_start(tri_sb[:], tri.ap().bitcast(F32R))
            eye_sb = cpool.tile([P, P], F32R, tag="eye")
            nc.sync.dma_start(eye_sb[:], eye.ap().bitcast(F32R))
            ones_sb = cpool.tile([P, P], F32R, tag="ones")
            nc.sync.dma_start(ones_sb[:], ones.ap().bitcast(F32R))
            ones1_sb = ones_sb[0:1, :]

            qT_sb = cpool.tile([P, T], F32R, tag="qT")
            kT_sb = cpool.tile([P, T], F32R, tag="kT")
            yT_sb = cpool.tile([P, T], F32R, tag="yT")
            # v in [t-part, (head0 d + one | head1 d + one)] layout
            v_sb = cpool.tile([P, n_kb, 2 * (D + 1)], F32R, tag="v")
            nc.vector.tensor_copy(
                v_sb[:, :, D], ones_sb[:, 0:1].to_broadcast((P, n_kb)))
            nc.vector.tensor_copy(
                v_sb[:, :, 2 * D + 1], ones_sb[:, 0:1].to_broadcast((P, n_kb)))

            # ---- q/k/v projections ----
            with (
                tc.tile_pool(name="xin", bufs=3) as xin,
                tc.tile_pool(name="proj_ps", bufs=2, space="PSUM") as pps,
                tc.tile_pool(name="tr_ps", bufs=2, space="PSUM") as trps,
            ):
                for ts in range(n_ts):
                    t0 = ts * TS
                    prj = [pps.tile([P, TS], F32, tag=f"prj{j}", name=f"prj{j}_{ts}") for j in range(3)]
                    for c in range(n_ct):
                        x_t = xin.tile([P, TS], F32R, tag="x")
                        nc.sync.dma_start(
                            x_t[:], xT_r[c * P:(c + 1) * P, t0:t0 + TS])
                        for j in range(3):
                            nc.tensor.matmul(
                                prj[j][:],
                                w3_sb[:, c, j * P:(j + 1) * P],
                                x_t[:],
                                start=(c == 0), stop=(c == n_ct - 1))
                    # bias add + copy out of PSUM
                    nc.vector.tensor_tensor(
                        qT_sb[:, t0:t0 + TS], prj[0][:],
                        b3_sb[:, 0:1].to_broadcast((P, TS)), ADD)
                    nc.vector.tensor_tensor(
                        kT_sb[:, t0:t0 + TS], prj[1][:],
                        b3_sb[:, 1:2].to_broadcast((P, TS)), ADD)
                    vt_t = xin.tile([P, TS], F32R, tag="vt")
                    nc.vector.tensor_tensor(
                        vt_t[:], prj[2][:],
                        b3_sb[:, 2:3].to_broadcast((P, TS)), ADD)
                    # transpose v chunk into [t, c_local] layout
                    for kk in range(TS // P):
                        gkb = (t0 + kk * P) // P
                        t_ps = trps.tile([P, P], F32R, tag="tr")
                        nc.tensor.transpose(
                            t_ps[:], vt_t[:, kk * P:(kk + 1) * P], eye_sb[:])
                        for h in range(2):
                            nc.vector.tensor_copy(
                                v_sb[:, gkb, h * (D + 1):h * (D + 1) + D],
                                t_ps[:, h * D:(h + 1) * D])

            # ---- attention + interleaved output projection ----
            with (
                tc.tile_pool(name="st_ps", bufs=2, space="PSUM") as stps,
                tc.tile_pool(name="y_ps", bufs=1, space="PSUM") as yps,
                tc.tile_pool(name="r_ps", bufs=1, space="PSUM") as rps,
                tc.tile_pool(name="o_ps", bufs=1, space="PSUM") as ops,
            ):
                attention_phase(nc, tc, stps, yps, rps, ops, psmall, ptp, osbp,
                                qT_sb, kT_sb, yT_sb, v_sb, tri_sb, ones1_sb,
                                wp_sb, partial, T)

    nc.compile()
    return nc


def attention_phase(nc, tc, stps, yps, rps, ops, psmall, ptp, osbp,
                    qT_sb, kT_sb, yT_sb, v_sb, tri_sb, ones1_sb,
                    wp_sb, partial, T):
            n_qt = T // QT
            kb_per_qt = QT // P
            for qt in range(n_qt):
                q0 = qt * QT
                nkb = (qt + 1) * kb_per_qt
                y_ps = [yps.tile([P, QT], F32, tag=f"y{h}", name=f"y{h}_{qt}") for h in range(2)]
                for kb in range(nkb):
                    diag = kb >= qt * kb_per_qt
                    c = (kb - qt * kb_per_qt) * P if diag else 0
                    for h in range(2):
                        s_ps = stps.tile([P, QT], F32, tag="s")
                        nc.tensor.matmul(
                            s_ps[:, c:QT],
                            kT_sb[D * h:D * h + D, kb * P:(kb + 1) * P],
                            qT_sb[D * h:D * h + D, q0 + c:q0 + QT],
                            start=True, stop=True)
                        p_sb = ptp.tile([P, QT], F32R, tag="p")
                        nc.scalar.activation(
                            p_sb[:, c:QT], s_ps[:, c:QT], EXP,
                            scale=1.0 / np.sqrt(D))
                        if diag:
                            nc.vector.tensor_tensor(
                                p_sb[:, c:c + P], p_sb[:, c:c + P],
                                tri_sb[:], MULT)
                        nc.tensor.matmul(
                            y_ps[h][0:D + 1, c:QT],
                            v_sb[:, kb, h * (D + 1):(h + 1) * (D + 1)],
                            p_sb[:, c:QT],
                            start=(kb == 0), stop=(kb == nkb - 1))
                # softmax normalization: y /= sums (row D of y_ps)
                r2 = psmall.tile([1, 2 * QT], F32R, tag="r2")
                with nc.allow_low_precision(
                        reason="float32r reciprocal feeds float32r matmul"):
                    for h in range(2):
                        nc.vector.reciprocal(r2[0:1, h * QT:(h + 1) * QT],
                                             y_ps[h][D:D + 1, :])
                R_ps = rps.tile([P, 2 * QT], F32, tag="R")
                for i in range(2):
                    nc.tensor.matmul(R_ps[:, i * QT:(i + 1) * QT],
                                     ones1_sb[:],
                                     r2[:, i * QT:(i + 1) * QT],
                                     start=True, stop=True)
                R_sb = psmall.tile([P, QT], F32, tag="Rs")
                for h in range(2):
                    nc.vector.tensor_copy(
                        R_sb[D * h:D * (h + 1), :],
                        R_ps[D * h:D * (h + 1), h * QT:(h + 1) * QT])
                for h in range(2):
                    nc.vector.tensor_tensor(
                        yT_sb[D * h:D * (h + 1), q0:q0 + QT],
                        y_ps[h][0:D, :], R_sb[D * h:D * (h + 1), :], MULT)

                # output projection for the 4 t-blocks of this q-tile
                for tb in range(qt * (QT // P), (qt + 1) * (QT // P)):
                    o_ps = [ops.tile([P, 512], F32, tag=f"o{i}", name=f"o{i}_{tb}") for i in range(2)]
                    o_sb = osbp.tile([P, N_EMBD], F32, tag="osb")
                    for i in range(2):
                        nc.tensor.matmul(
                            o_ps[i][:],
                            yT_sb[:, tb * P:(tb + 1) * P],
                            wp_sb[:, i * 512:(i + 1) * 512],
                            start=True, stop=True)
                        nc.vector.tensor_copy(
                            o_sb[:, i * 512:(i + 1) * 512], o_ps[i][:])
                    nc.sync.dma_start(
                        partial.ap()[tb * P:(tb + 1) * P, :], o_sb[:])


def make_core_inputs(x, wq, bq, wk, bk, wv, bv, wp):
    """Per-core input dicts (host-side sharding). x: [T, C] fp32."""
    T = x.shape[0]
    xT = np.ascontiguousarray(x.T)
    tri = (np.arange(P)[None, :] >= np.arange(P)[:, None]).astype(np.float32)
    eye = np.eye(P, dtype=np.float32)
    ones = np.ones((P, P), dtype=np.float32)
    in_maps = []
    for core in range(N_CORES):
        ch = slice(core * C_LOCAL, (core + 1) * C_LOCAL)
        w3T = np.ascontiguousarray(
            np.concatenate([wq[ch].T, wk[ch].T, wv[ch].T], axis=1))
        b3 = np.ascontiguousarray(
            np.stack([bq[ch], bk[ch], bv[ch]], axis=1))
        wpT = np.ascontiguousarray(wp[:, ch].T)
        in_maps.append({
            "xT": xT, "w3T": w3T, "b3": b3, "wpT": wpT,
            "tri": tri, "eye": eye, "ones": ones,
        })
    return in_maps


_NC_CACHE = {}


def _get_nc(T):
    if T not in _NC_CACHE:
        _NC_CACHE[T] = build(T)
    return _NC_CACHE[T]


def kernel(x, wq, bq, wk, bk, wv, bv, wp, bp, trace=False):
    x = np.asarray(x)
    B, T, C = x.shape
    in_maps = make_core_inputs(
        x[0], np.asarray(wq), np.asarray(bq), np.asarray(wk), np.asarray(bk),
        np.asarray(wv), np.asarray(bv), np.asarray(wp))
    nc = _get_nc(T)
    res = run_bass_kernel_spmd(nc, in_maps, core_ids=list(range(N_CORES)),
                               trace=trace)
    out = res.results[0]["partial"].copy()
    for c in range(1, N_CORES):
        out += res.results[c]["partial"]
    out += np.asarray(bp)[None, :]
    kernel.last_result = res
    return out[None].astype(np.float32)


# revision 14
# speedup vs baseline: 1.0598x; 1.0598x over previous
"""Causal self-attention (B=1, T=4096, C=1024, H=16) on 8 Trainium2 cores.

Sharding: tensor-parallel over heads. Each core owns 2 heads (128 channels):
  - computes q/k/v projections for its channels against the full sequence
    (x passed transposed as [C, T] so the contraction dim lands on partitions),
  - runs causal attention for its 2 heads in S^T = [k, q] layout
    (exp on ScalarE with fused 1/sqrt(D) scale; an appended ones-column in v
    yields softmax denominators as an extra output row of the same matmul),
  - multiplies its y slice against its 128 rows of wp -> a full [T, C] partial.
Host sums the 8 partials and adds the output bias. Identical SPMD program on
all cores; per-core data arrives via the input maps.

Dtype strategy (flags below): bf16 runs the PE at 1 cycle/row and keeps the
HAM clock-gate warm; float32r is 2 cycles/row but ~13-bit accurate. The
softmax normalizer path always stays float32r/fp32.
"""

import numpy as np
import ml_dtypes

import concourse.bass as bass
import concourse.mybir as mybir
import concourse.tile as tile
from concourse import bacc
from concourse.bass_utils import run_bass_kernel_spmd

P = 128
N_EMBD = 1024
N_HEAD = 16
T_FULL = 4096
N_CORES = 8
D = N_EMBD // N_HEAD          # 64
C_LOCAL = 2 * D               # 128 channels per core (2 heads)
F32 = mybir.dt.float32
F32R = mybir.dt.float32r
BF16 = mybir.dt.bfloat16
ADD = mybir.AluOpType.add
MULT = mybir.AluOpType.mult
EXP = mybir.ActivationFunctionType.Exp

QT = 512   # attention q-tile (matmul moving free dim)
TS = 512   # projection t-chunk

# dtype switches
PROJ_BF16 = False   # x / qkv-weight operands
ATT_BF16 = True     # qT/kT (S^T matmul) and P/v (y matmul) operands
OUT_BF16 = False    # yT / wp operands


def _np_dt(dt):
    return ml_dtypes.bfloat16 if dt == BF16 else np.float32


def build(T=T_FULL):
    n_ts = T // TS
    n_qt = T // QT
    n_kb = T // P
    n_ct = N_EMBD // P
    kb_per_qt = QT // P

    XW_DT = BF16 if PROJ_BF16 else F32R
    QK_DT = BF16 if ATT_BF16 else F32R
    PV_DT = BF16 if ATT_BF16 else F32R
    Y_DT = BF16 if OUT_BF16 else F32R

    nc = bacc.Bacc("TRN2", target_bir_lowering=False, debug=False,
                   num_devices=N_CORES)
    xdt = BF16 if PROJ_BF16 else F32
    xT = nc.dram_tensor("xT", [N_EMBD, T], xdt, kind="ExternalInput")
    w3T = nc.dram_tensor("w3T", [N_EMBD, 3 * C_LOCAL], xdt,
                         kind="ExternalInput")
    b3 = nc.dram_tensor("b3", [C_LOCAL, 3], F32, kind="ExternalInput")
    wpdt = BF16 if OUT_BF16 else F32
    wpT = nc.dram_tensor("wpT", [C_LOCAL, N_EMBD], wpdt, kind="ExternalInput")
    tdt = BF16 if ATT_BF16 else F32
    tri = nc.dram_tensor("tri", [P, P], tdt, kind="ExternalInput")
    eye = nc.dram_tensor("eye", [P, P], tdt, kind="ExternalInput")
    ones = nc.dram_tensor("ones", [P, P], F32, kind="ExternalInput")
    partial = nc.dram_tensor("partial", [T, N_EMBD], F32,
                             kind="ExternalOutput")

    def _in(ap, dt):
        return ap.bitcast(dt) if dt in (F32R,) else ap

    with tile.TileContext(nc) as tc:
        with (
            tc.tile_pool(name="const", bufs=1) as cpool,
            tc.tile_pool(name="psmall", bufs=2) as psmall,
            tc.tile_pool(name="pt", bufs=4) as ptp,
            tc.tile_pool(name="osb", bufs=2) as osbp,
        ):
            # ---- constants ----
            w3_sb = cpool.tile([P, n_ct, 3 * C_LOCAL], XW_DT, tag="w3")
            nc.sync.dma_start(
                w3_sb[:],
                _in(w3T.ap(), XW_DT).rearrange("(co p) o -> p co o", p=P))
            wp_sb = cpool.tile([P, N_EMBD], Y_DT, tag="wp")
            nc.sync.dma_start(wp_sb[:], _in(wpT.ap(), Y_DT))
            b3_sb = cpool.tile([P, 3], F32, tag="b3")
            nc.sync.dma_start(b3_sb[:], b3.ap())
            tri_sb = cpool.tile([P, P], PV_DT, tag="tri")
            nc.sync.dma_start(tri_sb[:], _in(tri.ap(), PV_DT))
            eye_sb = cpool.tile([P, P], PV_DT, tag="eye")
            nc.sync.dma_start(eye_sb[:], _in(eye.ap(), PV_DT))
            ones_sb = cpool.tile([P, P], F32R, tag="ones")
            nc.sync.dma_start(ones_sb[:], ones.ap().bitcast(F32R))
            ones1_sb = ones_sb[0:1, :]

            qT_sb = cpool.tile([P, T], QK_DT, tag="qT")
            kT_sb = cpool.tile([P, T], QK_DT, tag="kT")
            yT_sb = cpool.tile([P, T], Y_DT, tag="yT")
            # v in [t-part, (head0 d + one | head1 d + one)] layout
            v_sb = cpool.tile([P, n_kb, 2 * (D + 1)], PV_DT, tag="v")
            nc.vector.tensor_copy(
                v_sb[:, :, D], ones_sb[:, 0:1].to_broadcast((P, n_kb)))
            nc.vector.tensor_copy(
                v_sb[:, :, 2 * D + 1], ones_sb[:, 0:1].to_broadcast((P, n_kb)))

            # ---- q/k/v projections ----
            with (
                tc.tile_pool(name="xin", bufs=3) as xin,
                tc.tile_pool(name="proj_ps", bufs=2, space="PSUM") as pps,
                tc.tile_pool(name="tr_ps", bufs=2, space="PSUM") as trps,
            ):
                for ts in range(n_ts):
                    t0 = ts * TS
                    prj = [pps.tile([P, TS], F32, tag=f"prj{j}",
                                    name=f"prj{j}_{ts}") for j in range(3)]
                    for c in range(n_ct):
                        x_t = xin.tile([P, TS], XW_DT, tag="x")
                        nc.sync.dma_start(
                            x_t[:],
                            _in(xT.ap(), XW_DT)[c * P:(c + 1) * P, t0:t0 + TS])
                        for j in range(3):
                            nc.tensor.matmul(
                                prj[j][:],
                                w3_sb[:, c, j * P:(j + 1) * P],
                                x_t[:],
                                start=(c == 0), stop=(c == n_ct - 1))
                    # bias add + copy out of PSUM
                    nc.vector.tensor_tensor(
                        qT_sb[:, t0:t0 + TS], prj[0][:],
                        b3_sb[:, 0:1].to_broadcast((P, TS)), ADD)
                    nc.vector.tensor_tensor(
                        kT_sb[:, t0:t0 + TS], prj[1][:],
                        b3_sb[:, 1:2].to_broadcast((P, TS)), ADD)
                    vt_t = xin.tile([P, TS], PV_DT, tag="vt")
                    nc.vector.tensor_tensor(
                        vt_t[:], prj[2][:],
                        b3_sb[:, 2:3].to_broadcast((P, TS)), ADD)
                    # transpose v chunk into [t, c_local] layout
                    for kk in range(TS // P):
                        gkb = (t0 + kk * P) // P
                        t_ps = trps.tile([P, P], PV_DT, tag="tr")
                        nc.tensor.transpose(
                            t_ps[:], vt_t[:, kk * P:(kk + 1) * P], eye_sb[:])
                        for h in range(2):
                            nc.vector.tensor_copy(
                                v_sb[:, gkb, h * (D + 1):h * (D + 1) + D],
                                t_ps[:, h * D:(h + 1) * D])

            # ---- attention + interleaved output projection ----
            with (
                tc.tile_pool(name="st_ps", bufs=4, space="PSUM") as stps,
                tc.tile_pool(name="y_ps", bufs=1, space="PSUM") as yps,
                tc.tile_pool(name="r_ps", bufs=1, space="PSUM") as rps,
            ):
                for qt in range(n_qt):
                    q0 = qt * QT
                    nkb = (qt + 1) * kb_per_qt
                    y_ps = [yps.tile([P, QT], F32, tag=f"y{h}",
                                     name=f"y{h}_{qt}") for h in range(2)]

                    # software-pipelined: S/exp for kb, y-matmul for kb-1
                    prev_p = None

                    def emit_y(kb, p_pair, first, last):
                        dg = kb >= qt * kb_per_qt
                        cc = (kb - qt * kb_per_qt) * P if dg else 0
                        for h in range(2):
                            nc.tensor.matmul(
                                y_ps[h][0:D + 1, cc:QT],
                                v_sb[:, kb, h * (D + 1):(h + 1) * (D + 1)],
                                p_pair[h][:, cc:QT],
                                start=first, stop=last)

                    for kb in range(nkb):
                        diag = kb >= qt * kb_per_qt
                        c = (kb - qt * kb_per_qt) * P if diag else 0
                        cur_p = []
                        for h in range(2):
                            s_ps = stps.tile([P, QT], F32, tag="s")
                            nc.tensor.matmul(
                                s_ps[:, c:QT],
                                kT_sb[D * h:D * h + D, kb * P:(kb + 1) * P],
                                qT_sb[D * h:D * h + D, q0 + c:q0 + QT],
                                start=True, stop=True)
                            p_sb = ptp.tile([P, QT], PV_DT, tag="p")
                            nc.scalar.activation(
                                p_sb[:, c:QT], s_ps[:, c:QT], EXP,
                                scale=1.0 / float(np.sqrt(D)))
                            if diag:
                                nc.vector.tensor_tensor(
                                    p_sb[:, c:c + P], p_sb[:, c:c + P],
                                    tri_sb[:], MULT)
                            cur_p.append(p_sb)
                        if prev_p is not None:
                            emit_y(kb - 1, prev_p, kb - 1 == 0, False)
                        prev_p = cur_p
                    emit_y(nkb - 1, prev_p, nkb - 1 == 0, True)

                    # softmax normalization: y /= sums (row D of y_ps)
                    s2 = psmall.tile([1, 2 * QT], F32R, tag="s2")
                    for h in range(2):
                        nc.scalar.copy(s2[0:1, h * QT:(h + 1) * QT],
                                       y_ps[h][D:D + 1, :])
                    R_ps = rps.tile([P, 2 * QT], F32, tag="R")
                    for i in range(2):
                        nc.tensor.matmul(
                            R_ps[:, i * QT:(i + 1) * QT],
                            ones1_sb,
                            s2[:, i * QT:(i + 1) * QT],
                            start=True, stop=True)
                    R_sb = psmall.tile([P, 2 * QT], F32, tag="Rs")
                    with nc.allow_low_precision(reason="softmax normalizer"):
                        nc.vector.reciprocal(R_sb[:], R_ps[:])
                    for h in range(2):
                        nc.vector.tensor_tensor(
                            yT_sb[D * h:D * (h + 1), q0:q0 + QT],
                            y_ps[h][0:D, :],
                            R_sb[D * h:D * (h + 1), h * QT:(h + 1) * QT],
                            MULT)

                    # output projection for the 4 t-blocks of this q-tile
                    for tb in range(qt * (QT // P), (qt + 1) * (QT // P)):
                        o_ps = [yps.tile([P, 512], F32, tag=f"y{i}",
                                         name=f"o{i}_{tb}") for i in range(2)]
                        o_sb = osbp.tile([P, N_EMBD], F32, tag="osb")
                        for i in range(2):
                            nc.tensor.matmul(
                                o_ps[i][:],
                                yT_sb[:, tb * P:(tb + 1) * P],
                                wp_sb[:, i * 512:(i + 1) * 512],
                                start=True, stop=True)
                            nc.vector.tensor_copy(
                                o_sb[:, i * 512:(i + 1) * 512], o_ps[i][:])
                        nc.sync.dma_start(
                            partial.ap()[tb * P:(tb + 1) * P, :], o_sb[:])

    nc.compile()
    return nc


def make_core_inputs(x, wq, bq, wk, bk, wv, bv, wp):
    """Per-core input dicts (host-side sharding). x: [T, C] fp32."""
    xw_np = _np_dt(BF16 if PROJ_BF16 else F32)
    y_np = _np_dt(BF16 if OUT_BF16 else F32)
    t_np = _np_dt(BF16 if ATT_BF16 else F32)
    xT = np.ascontiguousarray(x.T).astype(xw_np)
    tri = (np.arange(P)[None, :] >= np.arange(P)[:, None]).astype(t_np)
    eye = np.eye(P, dtype=np.float32).astype(t_np)
    ones = np.ones((P, P), dtype=np.float32)
    in_maps = []
    for core in range(N_CORES):
        ch = slice(core * C_LOCAL, (core + 1) * C_LOCAL)
        w3T = np.ascontiguousarray(
            np.concatenate([wq[ch].T, wk[ch].T, wv[ch].T], axis=1)
        ).astype(xw_np)
        b3 = np.ascontiguousarray(
            np.stack([bq[ch], bk[ch], bv[ch]], axis=1)).astype(np.float32)
        wpT = np.ascontiguousarray(wp[:, ch].T).astype(y_np)
        in_maps.append({
            "xT": xT, "w3T": w3T, "b3": b3, "wpT": wpT,
            "tri": tri, "eye": eye, "ones": ones,
        })
    return in_maps


_NC_CACHE = {}


def _get_nc(T):
    if T not in _NC_CACHE:
        _NC_CACHE[T] = build(T)
    return _NC_CACHE[T]


def kernel(x, wq, bq, wk, bk, wv, bv, wp, bp, trace=False):
    x = np.asarray(x)
    B, T, C = x.shape
    in_maps = make_core_inputs(
        x[0], np.asarray(wq), np.asarray(bq), np.asarray(wk), np.asarray(bk),
        np.asarray(wv), np.asarray(bv), np.asarray(wp))
    nc = _get_nc(T)
    res = run_bass_kernel_spmd(nc, in_maps, core_ids=list(range(N_CORES)),
                               trace=trace)
    out = res.results[0]["partial"].copy()
    for c in range(1, N_CORES):
        out += res.results[c]["partial"]
    out += np.asarray(bp)[None, :]
    kernel.last_result = res
    return out[None].astype(np.float32)


# revision 16
# speedup vs baseline: 1.2902x; 1.2174x over previous
"""Causal self-attention (B=1, T=4096, C=1024, H=16) on 8 Trainium2 cores.

Sharding: tensor-parallel over heads. Each core owns 2 heads (128 channels):
  - computes q/k/v projections for its channels against the full sequence
    (x passed transposed as [C, T] so the contraction dim lands on partitions),
  - runs causal attention for its 2 heads in S^T = [k, q] layout
    (exp on ScalarE with fused 1/sqrt(D) scale; an appended ones-column in v
    yields softmax denominators as an extra output row of the same matmul),
  - multiplies its y slice against its 128 rows of wp -> a full [T, C] partial.
Host sums the 8 partials and adds the output bias. Identical SPMD program on
all cores; per-core data arrives via the input maps.

Dtype strategy (flags below): bf16 runs the PE at 1 cycle/row and keeps the
HAM clock-gate warm; float32r is 2 cycles/row but ~13-bit accurate. The
softmax normalizer path always stays float32r/fp32.
"""

import numpy as np
import ml_dtypes

import concourse.bass as bass
import concourse.mybir as mybir
import concourse.tile as tile
from concourse import bacc
from concourse.bass_utils import run_bass_kernel_spmd

P = 128
N_EMBD = 1024
N_HEAD = 16
T_FULL = 4096
N_CORES = 8
D = N_EMBD // N_HEAD          # 64
C_LOCAL = 2 * D               # 128 channels per core (2 heads)
F32 = mybir.dt.float32
F32R = mybir.dt.float32r
BF16 = mybir.dt.bfloat16
ADD = mybir.AluOpType.add
MULT = mybir.AluOpType.mult
EXP = mybir.ActivationFunctionType.Exp

QT = 512   # attention q-tile (matmul moving free dim)
TS = 512   # projection t-chunk

# dtype switches
PROJ_BF16 = False   # x / qkv-weight operands
ATT_BF16 = True     # qT/kT (S^T matmul) and P/v (y matmul) operands
OUT_BF16 = False    # yT / wp operands


def _np_dt(dt):
    return ml_dtypes.bfloat16 if dt == BF16 else np.float32


def build(T=T_FULL):
    n_ts = T // TS
    n_qt = T // QT
    n_kb = T // P
    n_ct = N_EMBD // P
    kb_per_qt = QT // P

    XW_DT = BF16 if PROJ_BF16 else F32R
    QK_DT = BF16 if ATT_BF16 else F32R
    PV_DT = BF16 if ATT_BF16 else F32R
    Y_DT = BF16 if OUT_BF16 else F32R

    nc = bacc.Bacc("TRN2", target_bir_lowering=False, debug=False,
                   num_devices=N_CORES)
    xdt = BF16 if PROJ_BF16 else F32
    xT = nc.dram_tensor("xT", [N_EMBD, T], xdt, kind="ExternalInput")
    w3T = nc.dram_tensor("w3T", [N_EMBD, 3 * C_LOCAL], xdt,
                         kind="ExternalInput")
    b3 = nc.dram_tensor("b3", [C_LOCAL, 3], F32, kind="ExternalInput")
    wpdt = BF16 if OUT_BF16 else F32
    wpT = nc.dram_tensor("wpT", [C_LOCAL, N_EMBD], wpdt, kind="ExternalInput")
    tdt = BF16 if ATT_BF16 else F32
    tri = nc.dram_tensor("tri", [P, P], tdt, kind="ExternalInput")
    eye = nc.dram_tensor("eye", [P, P], tdt, kind="ExternalInput")
    ones = nc.dram_tensor("ones", [P, P], F32, kind="ExternalInput")
    partial = nc.dram_tensor("partial", [T, N_EMBD], F32,
                             kind="ExternalOutput")

    def _in(ap, dt):
        return ap.bitcast(dt) if dt in (F32R,) else ap

    with tile.TileContext(nc) as tc:
        with (
            tc.tile_pool(name="const", bufs=1) as cpool,
            tc.tile_pool(name="psmall", bufs=2) as psmall,
            tc.tile_pool(name="pt", bufs=4) as ptp,
            tc.tile_pool(name="osb", bufs=2) as osbp,
        ):
            # ---- constants ----
            w3_sb = cpool.tile([P, n_ct, 3 * C_LOCAL], XW_DT, tag="w3")
            nc.sync.dma_start(
                w3_sb[:],
                _in(w3T.ap(), XW_DT).rearrange("(co p) o -> p co o", p=P))
            wp_sb = cpool.tile([P, N_EMBD], Y_DT, tag="wp")
            nc.sync.dma_start(wp_sb[:], _in(wpT.ap(), Y_DT))
            b3_sb = cpool.tile([P, 3], F32, tag="b3")
            nc.sync.dma_start(b3_sb[:], b3.ap())
            tri_sb = cpool.tile([P, P], PV_DT, tag="tri")
            nc.sync.dma_start(tri_sb[:], _in(tri.ap(), PV_DT))
            eye_sb = cpool.tile([P, P], PV_DT, tag="eye")
            nc.sync.dma_start(eye_sb[:], _in(eye.ap(), PV_DT))
            ones_sb = cpool.tile([P, P], F32R, tag="ones")
            nc.sync.dma_start(ones_sb[:], ones.ap().bitcast(F32R))
            ones1_sb = ones_sb[0:1, :]

            qT_sb = cpool.tile([P, T], QK_DT, tag="qT")
            kT_sb = cpool.tile([P, T], QK_DT, tag="kT")
            yT_sb = cpool.tile([P, T], Y_DT, tag="yT")
            # v in [t-part, (head0 d + one | head1 d + one)] layout
            v_sb = cpool.tile([P, n_kb, 2 * (D + 1)], PV_DT, tag="v")
            nc.vector.tensor_copy(
                v_sb[:, :, D], ones_sb[:, 0:1].to_broadcast((P, n_kb)))
            nc.vector.tensor_copy(
                v_sb[:, :, 2 * D + 1], ones_sb[:, 0:1].to_broadcast((P, n_kb)))

            # ---- q/k/v projections ----
            with (
                tc.tile_pool(name="xin", bufs=3) as xin,
                tc.tile_pool(name="proj_ps", bufs=2, space="PSUM") as pps,
                tc.tile_pool(name="tr_ps", bufs=2, space="PSUM") as trps,
            ):
                for ts in range(n_ts):
                    t0 = ts * TS
                    prj = [pps.tile([P, TS], F32, tag=f"prj{j}",
                                    name=f"prj{j}_{ts}") for j in range(3)]
                    for c in range(n_ct):
                        x_t = xin.tile([P, TS], XW_DT, tag="x")
                        nc.sync.dma_start(
                            x_t[:],
                            _in(xT.ap(), XW_DT)[c * P:(c + 1) * P, t0:t0 + TS])
                        for j in range(3):
                            nc.tensor.matmul(
                                prj[j][:],
                                w3_sb[:, c, j * P:(j + 1) * P],
                                x_t[:],
                                start=(c == 0), stop=(c == n_ct - 1))
                    # bias add + copy out of PSUM
                    nc.vector.tensor_tensor(
                        qT_sb[:, t0:t0 + TS], prj[0][:],
                        b3_sb[:, 0:1].to_broadcast((P, TS)), ADD)
                    nc.vector.tensor_tensor(
                        kT_sb[:, t0:t0 + TS], prj[1][:],
                        b3_sb[:, 1:2].to_broadcast((P, TS)), ADD)
                    vt_t = xin.tile([P, TS], PV_DT, tag="vt")
                    nc.vector.tensor_tensor(
                        vt_t[:], prj[2][:],
                        b3_sb[:, 2:3].to_broadcast((P, TS)), ADD)
                    # transpose v chunk into [t, c_local] layout
                    for kk in range(TS // P):
                        gkb = (t0 + kk * P) // P
                        t_ps = trps.tile([P, P], PV_DT, tag="tr")
                        nc.tensor.transpose(
                            t_ps[:], vt_t[:, kk * P:(kk + 1) * P], eye_sb[:])
                        for h in range(2):
                            nc.vector.tensor_copy(
                                v_sb[:, gkb, h * (D + 1):h * (D + 1) + D],
                                t_ps[:, h * D:(h + 1) * D])

            # ---- attention + interleaved output projection ----
            with (
                tc.tile_pool(name="st_ps", bufs=2, space="PSUM") as stps,
                tc.tile_pool(name="y_ps", bufs=1, space="PSUM") as yps,
                tc.tile_pool(name="o_ps", bufs=1, space="PSUM") as ops,
            ):
                def emit_outproj(qt):
                    # output projection for the 4 t-blocks of q-tile qt
                    for tb in range(qt * (QT // P), (qt + 1) * (QT // P)):
                        o_ps = [ops.tile([P, 512], F32, tag=f"o{i}",
                                         name=f"o{i}_{tb}") for i in range(2)]
                        o_sb = osbp.tile([P, N_EMBD], F32, tag="osb")
                        for i in range(2):
                            nc.tensor.matmul(
                                o_ps[i][:],
                                yT_sb[:, tb * P:(tb + 1) * P],
                                wp_sb[:, i * 512:(i + 1) * 512],
                                start=True, stop=True)
                            nc.vector.tensor_copy(
                                o_sb[:, i * 512:(i + 1) * 512], o_ps[i][:])
                        nc.sync.dma_start(
                            partial.ap()[tb * P:(tb + 1) * P, :], o_sb[:])

                pending = None
                for qt in range(n_qt):
                    q0 = qt * QT
                    nkb = (qt + 1) * kb_per_qt
                    y_ps = [yps.tile([P, QT], F32, tag=f"y{h}",
                                     name=f"y{h}_{qt}") for h in range(2)]

                    # software-pipelined: S/exp for kb, y-matmul for kb-1
                    prev_p = None

                    def emit_y(kb, p_pair, first, last):
                        dg = kb >= qt * kb_per_qt
                        cc = (kb - qt * kb_per_qt) * P if dg else 0
                        for h in range(2):
                            nc.tensor.matmul(
                                y_ps[h][0:D + 1, cc:QT],
                                v_sb[:, kb, h * (D + 1):(h + 1) * (D + 1)],
                                p_pair[:, h, cc:QT],
                                start=first, stop=last)

                    for kb in range(nkb):
                        diag = kb >= qt * kb_per_qt
                        c = (kb - qt * kb_per_qt) * P if diag else 0
                        # both heads' scores in one 2-bank tile -> one exp
                        s_pair = stps.tile([P, 2, QT], F32, tag="s",
                                           name=f"s_{qt}_{kb}")
                        for h in range(2):
                            nc.tensor.matmul(
                                s_pair[:, h, c:QT],
                                kT_sb[D * h:D * h + D, kb * P:(kb + 1) * P],
                                qT_sb[D * h:D * h + D, q0 + c:q0 + QT],
                                start=True, stop=True)
                        p_pair = ptp.tile([P, 2, QT], PV_DT, tag="p",
                                          name=f"p_{qt}_{kb}")
                        nc.scalar.activation(
                            p_pair[:, :, c:QT], s_pair[:, :, c:QT], EXP,
                            scale=1.0 / float(np.sqrt(D)))
                        if diag:
                            for h in range(2):
                                nc.vector.tensor_tensor(
                                    p_pair[:, h, c:c + P], p_pair[:, h, c:c + P],
                                    tri_sb[:], MULT)
                        if kb == 2 and pending is not None:
                            emit_outproj(pending)
                            pending = None
                        if prev_p is not None:
                            emit_y(kb - 1, prev_p, kb - 1 == 0, False)
                        prev_p = p_pair
                    emit_y(nkb - 1, prev_p, nkb - 1 == 0, True)

                    # softmax normalization: y /= sums (row D of y_ps)
                    s2 = psmall.tile([1, 2 * QT], F32R, tag="s2")
                    for h in range(2):
                        nc.scalar.copy(s2[0:1, h * QT:(h + 1) * QT],
                                       y_ps[h][D:D + 1, :])
                    R_ps = stps.tile([P, 2, QT], F32, tag="s", name=f"R_{qt}")
                    for i in range(2):
                        nc.tensor.matmul(
                            R_ps[:, i, :],
                            ones1_sb,
                            s2[:, i * QT:(i + 1) * QT],
                            start=True, stop=True)
                    R_sb = psmall.tile([P, 2, QT], F32, tag="Rs")
                    with nc.allow_low_precision(reason="softmax normalizer"):
                        nc.vector.reciprocal(R_sb[:], R_ps[:])
                    for h in range(2):
                        nc.vector.tensor_tensor(
                            yT_sb[D * h:D * (h + 1), q0:q0 + QT],
                            y_ps[h][0:D, :],
                            R_sb[D * h:D * (h + 1), h, :],
                            MULT)
                    if pending is not None:
                        emit_outproj(pending)
                    pending = qt
                emit_outproj(pending)

    nc.compile()
    return nc


def make_core_inputs(x, wq, bq, wk, bk, wv, bv, wp):
    """Per-core input dicts (host-side sharding). x: [T, C] fp32."""
    xw_np = _np_dt(BF16 if PROJ_BF16 else F32)
    y_np = _np_dt(BF16 if OUT_BF16 else F32)
    t_np = _np_dt(BF16 if ATT_BF16 else F32)
    xT = np.ascontiguousarray(x.T).astype(xw_np)
    tri = (np.arange(P)[None, :] >= np.arange(P)[:, None]).astype(t_np)
    eye = np.eye(P, dtype=np.float32).astype(t_np)
    ones = np.ones((P, P), dtype=np.float32)
    in_maps = []
    for core in range(N_CORES):
        ch = slice(core * C_LOCAL, (core + 1) * C_LOCAL)
        w3T = np.ascontiguousarray(
            np.concatenate([wq[ch].T, wk[ch].T, wv[ch].T], axis=1)
        ).astype(xw_np)
        b3 = np.ascontiguousarray(
            np.stack([bq[ch], bk[ch], bv[ch]], axis=1)).astype(np.float32)
        wpT = np.ascontiguousarray(wp[:, ch].T).astype(y_np)
        in_maps.append({
            "xT": xT, "w3T": w3T, "b3": b3, "wpT": wpT,
            "tri": tri, "eye": eye, "ones": ones,
        })
    return in_maps


_NC_CACHE = {}


def _get_nc(T):
    if T not in _NC_CACHE:
        _NC_CACHE[T] = build(T)
    return _NC_CACHE[T]


def kernel(x, wq, bq, wk, bk, wv, bv, wp, bp, trace=False):
    x = np.asarray(x)
    B, T, C = x.shape
    in_maps = make_core_inputs(
        x[0], np.asarray(wq), np.asarray(bq), np.asarray(wk), np.asarray(bk),
        np.asarray(wv), np.asarray(bv), np.asarray(wp))
    nc = _get_nc(T)
    res = run_bass_kernel_spmd(nc, in_maps, core_ids=list(range(N_CORES)),
                               trace=trace)
    out = res.results[0]["partial"].copy()
    for c in range(1, N_CORES):
        out += res.results[c]["partial"]
    out += np.asarray(bp)[None, :]
    kernel.last_result = res
    return out[None].astype(np.float32)


# revision 17
# speedup vs baseline: 1.4094x; 1.0924x over previous
"""Causal self-attention (B=1, T=4096, C=1024, H=16) on 8 Trainium2 cores.

Sharding: tensor-parallel over heads. Each core owns 2 heads (128 channels):
  - computes q/k/v projections for its channels against the full sequence
    (x passed transposed as [C, T] so the contraction dim lands on partitions),
  - runs causal attention for its 2 heads in S^T = [k, q] layout
    (exp on ScalarE with fused 1/sqrt(D) scale; an appended ones-column in v
    yields softmax denominators as an extra output row of the same matmul),
  - multiplies its y slice against its 128 rows of wp -> a full [T, C] partial.
Host sums the 8 partials and adds the output bias. Identical SPMD program on
all cores; per-core data arrives via the input maps.

Dtype strategy (flags below): bf16 runs the PE at 1 cycle/row and keeps the
HAM clock-gate warm; float32r is 2 cycles/row but ~13-bit accurate. The
softmax normalizer path always stays float32r/fp32.
"""

import numpy as np
import ml_dtypes

import concourse.bass as bass
import concourse.mybir as mybir
import concourse.tile as tile
from concourse import bacc
from concourse.bass_utils import run_bass_kernel_spmd

P = 128
N_EMBD = 1024
N_HEAD = 16
T_FULL = 4096
N_CORES = 8
D = N_EMBD // N_HEAD          # 64
C_LOCAL = 2 * D               # 128 channels per core (2 heads)
F32 = mybir.dt.float32
F32R = mybir.dt.float32r
BF16 = mybir.dt.bfloat16
ADD = mybir.AluOpType.add
MULT = mybir.AluOpType.mult
EXP = mybir.ActivationFunctionType.Exp
LOG = mybir.ActivationFunctionType.Ln if hasattr(mybir.ActivationFunctionType, "Ln") else mybir.ActivationFunctionType.Log
Y_LAG = 3  # y-matmul trails S/exp by this many k-blocks

QT = 512   # attention q-tile (matmul moving free dim)
TS = 512   # projection t-chunk

# dtype switches
PROJ_BF16 = True   # x / qkv-weight operands
ATT_BF16 = True     # qT/kT (S^T matmul) and P/v (y matmul) operands
OUT_BF16 = True    # yT / wp operands


def _np_dt(dt):
    return ml_dtypes.bfloat16 if dt == BF16 else np.float32


def build(T=T_FULL):
    n_ts = T // TS
    n_qt = T // QT
    n_kb = T // P
    n_ct = N_EMBD // P
    kb_per_qt = QT // P

    XW_DT = BF16 if PROJ_BF16 else F32R
    QK_DT = BF16 if ATT_BF16 else F32R
    PV_DT = BF16 if ATT_BF16 else F32R
    Y_DT = BF16 if OUT_BF16 else F32R

    nc = bacc.Bacc("TRN2", target_bir_lowering=False, debug=False,
                   num_devices=N_CORES)
    xdt = BF16 if PROJ_BF16 else F32
    xT = nc.dram_tensor("xT", [N_EMBD, T], xdt, kind="ExternalInput")
    w3T = nc.dram_tensor("w3T", [N_EMBD, 3 * C_LOCAL], xdt,
                         kind="ExternalInput")
    b3 = nc.dram_tensor("b3", [C_LOCAL, 3], F32, kind="ExternalInput")
    wpdt = BF16 if OUT_BF16 else F32
    wpT = nc.dram_tensor("wpT", [C_LOCAL, N_EMBD], wpdt, kind="ExternalInput")
    tdt = BF16 if ATT_BF16 else F32
    tri = nc.dram_tensor("tri", [P, P], tdt, kind="ExternalInput")
    eye = nc.dram_tensor("eye", [P, P], tdt, kind="ExternalInput")
    ones = nc.dram_tensor("ones", [P, P], F32, kind="ExternalInput")
    partial = nc.dram_tensor("partial", [T, N_EMBD], F32,
                             kind="ExternalOutput")

    def _in(ap, dt):
        return ap.bitcast(dt) if dt in (F32R,) else ap

    with tile.TileContext(nc) as tc:
        with (
            tc.tile_pool(name="const", bufs=1) as cpool,
            tc.tile_pool(name="psmall", bufs=2) as psmall,
            tc.tile_pool(name="pt", bufs=4) as ptp,
            tc.tile_pool(name="osb", bufs=2) as osbp,
        ):
            # ---- constants ----
            w3_sb = cpool.tile([P, n_ct, 3 * C_LOCAL], XW_DT, tag="w3")
            nc.sync.dma_start(
                w3_sb[:],
                _in(w3T.ap(), XW_DT).rearrange("(co p) o -> p co o", p=P))
            wp_sb = cpool.tile([P, N_EMBD], Y_DT, tag="wp")
            nc.sync.dma_start(wp_sb[:], _in(wpT.ap(), Y_DT))
            b3_sb = cpool.tile([P, 3], F32, tag="b3")
            nc.sync.dma_start(b3_sb[:], b3.ap())
            tri_sb = cpool.tile([P, P], PV_DT, tag="tri")
            nc.sync.dma_start(tri_sb[:], _in(tri.ap(), PV_DT))
            eye_sb = cpool.tile([P, P], PV_DT, tag="eye")
            nc.sync.dma_start(eye_sb[:], _in(eye.ap(), PV_DT))
            ones_sb = cpool.tile([P, P], F32R, tag="ones")
            nc.sync.dma_start(ones_sb[:], ones.ap().bitcast(F32R))
            ones1_sb = ones_sb[0:1, :]

            qT_sb = cpool.tile([P, T], QK_DT, tag="qT")
            kT_sb = cpool.tile([P, T], QK_DT, tag="kT")
            yT_sb = cpool.tile([P, T], Y_DT, tag="yT")
            # v in [t-part, (head0 d + one | head1 d + one)] layout
            v_sb = cpool.tile([P, n_kb, 2 * (D + 1)], PV_DT, tag="v")
            nc.vector.tensor_copy(
                v_sb[:, :, D], ones_sb[:, 0:1].to_broadcast((P, n_kb)))
            nc.vector.tensor_copy(
                v_sb[:, :, 2 * D + 1], ones_sb[:, 0:1].to_broadcast((P, n_kb)))

            # ---- q/k/v projections ----
            with (
                tc.tile_pool(name="xin", bufs=3) as xin,
                tc.tile_pool(name="proj_ps", bufs=2, space="PSUM") as pps,
                tc.tile_pool(name="tr_ps", bufs=2, space="PSUM") as trps,
            ):
                for ts in range(n_ts):
                    t0 = ts * TS
                    prj = [pps.tile([P, TS], F32, tag=f"prj{j}",
                                    name=f"prj{j}_{ts}") for j in range(3)]
                    for c in range(n_ct):
                        x_t = xin.tile([P, TS], XW_DT, tag="x")
                        nc.sync.dma_start(
                            x_t[:],
                            _in(xT.ap(), XW_DT)[c * P:(c + 1) * P, t0:t0 + TS])
                        for j in range(3):
                            nc.tensor.matmul(
                                prj[j][:],
                                w3_sb[:, c, j * P:(j + 1) * P],
                                x_t[:],
                                start=(c == 0), stop=(c == n_ct - 1))
                    # bias add + copy out of PSUM
                    nc.vector.tensor_tensor(
                        qT_sb[:, t0:t0 + TS], prj[0][:],
                        b3_sb[:, 0:1].to_broadcast((P, TS)), ADD)
                    nc.vector.tensor_tensor(
                        kT_sb[:, t0:t0 + TS], prj[1][:],
                        b3_sb[:, 1:2].to_broadcast((P, TS)), ADD)
                    vt_t = xin.tile([P, TS], PV_DT, tag="vt")
                    nc.vector.tensor_tensor(
                        vt_t[:], prj[2][:],
                        b3_sb[:, 2:3].to_broadcast((P, TS)), ADD)
                    # transpose v chunk into [t, c_local] layout
                    for kk in range(TS // P):
                        gkb = (t0 + kk * P) // P
                        t_ps = trps.tile([P, P], PV_DT, tag="tr")
                        nc.tensor.transpose(
                            t_ps[:], vt_t[:, kk * P:(kk + 1) * P], eye_sb[:])
                        for h in range(2):
                            nc.vector.tensor_copy(
                                v_sb[:, gkb, h * (D + 1):h * (D + 1) + D],
                                t_ps[:, h * D:(h + 1) * D])

            # ---- attention + interleaved output projection ----
            with (
                tc.tile_pool(name="st_ps", bufs=2, space="PSUM") as stps,
                tc.tile_pool(name="y_ps", bufs=1, space="PSUM") as yps,
                tc.tile_pool(name="o_ps", bufs=1, space="PSUM") as ops,
            ):
                def emit_outproj(qt):
                    # output projection for the 4 t-blocks of q-tile qt
                    for tb in range(qt * (QT // P), (qt + 1) * (QT // P)):
                        o_ps = [ops.tile([P, 512], F32, tag=f"o{i}",
                                         name=f"o{i}_{tb}") for i in range(2)]
                        o_sb = osbp.tile([P, N_EMBD], F32, tag="osb")
                        for i in range(2):
                            nc.tensor.matmul(
                                o_ps[i][:],
                                yT_sb[:, tb * P:(tb + 1) * P],
                                wp_sb[:, i * 512:(i + 1) * 512],
                                start=True, stop=True)
                            nc.vector.tensor_copy(
                                o_sb[:, i * 512:(i + 1) * 512], o_ps[i][:])
                        nc.sync.dma_start(
                            partial.ap()[tb * P:(tb + 1) * P, :], o_sb[:])

                pending = None
                for qt in range(n_qt):
                    q0 = qt * QT
                    nkb = (qt + 1) * kb_per_qt
                    y_ps = [yps.tile([P, QT], F32, tag=f"y{h}",
                                     name=f"y{h}_{qt}") for h in range(2)]

                    # software-pipelined: y-matmul trails S/exp by Y_LAG
                    from collections import deque
                    pend_y = deque()

                    def emit_y(kb, p_pair, first, last):
                        dg = kb >= qt * kb_per_qt
                        cc = (kb - qt * kb_per_qt) * P if dg else 0
                        for h in range(2):
                            nc.tensor.matmul(
                                y_ps[h][0:D + 1, cc:QT],
                                v_sb[:, kb, h * (D + 1):(h + 1) * (D + 1)],
                                p_pair[:, h, cc:QT],
                                start=first, stop=last)

                    for kb in range(nkb):
                        diag = kb >= qt * kb_per_qt
                        c = (kb - qt * kb_per_qt) * P if diag else 0
                        # both heads' scores in one 2-bank tile -> one exp
                        s_pair = stps.tile([P, 2, QT], F32, tag="s",
                                           name=f"s_{qt}_{kb}")
                        for h in range(2):
                            nc.tensor.matmul(
                                s_pair[:, h, c:QT],
                                kT_sb[D * h:D * h + D, kb * P:(kb + 1) * P],
                                qT_sb[D * h:D * h + D, q0 + c:q0 + QT],
                                start=True, stop=True)
                        p_pair = ptp.tile([P, 2, QT], PV_DT, tag="p",
                                          name=f"p_{qt}_{kb}")
                        nc.scalar.activation(
                            p_pair[:, :, c:QT], s_pair[:, :, c:QT], EXP,
                            scale=1.0 / float(np.sqrt(D)))
                        if diag:
                            for h in range(2):
                                nc.vector.tensor_tensor(
                                    p_pair[:, h, c:c + P], p_pair[:, h, c:c + P],
                                    tri_sb[:], MULT)
                        if kb == 2 and pending is not None:
                            emit_outproj(pending)
                            pending = None
                        pend_y.append((kb, p_pair))
                        if len(pend_y) > Y_LAG:
                            ykb, yp = pend_y.popleft()
                            emit_y(ykb, yp, ykb == 0, False)
                    while pend_y:
                        ykb, yp = pend_y.popleft()
                        emit_y(ykb, yp, ykb == 0, ykb == nkb - 1)

                    # softmax normalization: y *= 1/sums, via exp(-ln(s))
                    # (DVE reciprocal is ~6 cyc/elem; ln+exp on ScalarE is 1)
                    s2ln = psmall.tile([1, 2 * QT], F32, tag="s2")
                    for h in range(2):
                        nc.scalar.activation(s2ln[0:1, h * QT:(h + 1) * QT],
                                             y_ps[h][D:D + 1, :], LOG)
                    r2 = psmall.tile([1, 2 * QT], F32R, tag="r2")
                    nc.scalar.activation(r2[:], s2ln[:], EXP, scale=-1.0)
                    R_sb = psmall.tile([P, 2, QT], F32, tag="Rs")
                    for h in range(2):
                        R_h = ops.tile([P, QT], F32, tag=f"o{h}",
                                       name=f"R{h}_{qt}")
                        nc.tensor.matmul(
                            R_h[:], ones1_sb, r2[:, h * QT:(h + 1) * QT],
                            start=True, stop=True)
                        nc.vector.tensor_copy(
                            R_sb[D * h:D * (h + 1), h, :],
                            R_h[D * h:D * (h + 1), :])
                        nc.vector.tensor_tensor(
                            yT_sb[D * h:D * (h + 1), q0:q0 + QT],
                            y_ps[h][0:D, :],
                            R_sb[D * h:D * (h + 1), h, :],
                            MULT)
                    if pending is not None:
                        emit_outproj(pending)
                    pending = qt
                emit_outproj(pending)

    nc.compile()
    return nc


def make_core_inputs(x, wq, bq, wk, bk, wv, bv, wp):
    """Per-core input dicts (host-side sharding). x: [T, C] fp32."""
    xw_np = _np_dt(BF16 if PROJ_BF16 else F32)
    y_np = _np_dt(BF16 if OUT_BF16 else F32)
    t_np = _np_dt(BF16 if ATT_BF16 else F32)
    xT = np.ascontiguousarray(x.T).astype(xw_np)
    tri = (np.arange(P)[None, :] >= np.arange(P)[:, None]).astype(t_np)
    eye = np.eye(P, dtype=np.float32).astype(t_np)
    ones = np.ones((P, P), dtype=np.float32)
    in_maps = []
    for core in range(N_CORES):
        ch = slice(core * C_LOCAL, (core + 1) * C_LOCAL)
        w3T = np.ascontiguousarray(
            np.concatenate([wq[ch].T, wk[ch].T, wv[ch].T], axis=1)
        ).astype(xw_np)
        b3 = np.ascontiguousarray(
            np.stack([bq[ch], bk[ch], bv[ch]], axis=1)).astype(np.float32)
        wpT = np.ascontiguousarray(wp[:, ch].T).astype(y_np)
        in_maps.append({
            "xT": xT, "w3T": w3T, "b3": b3, "wpT": wpT,
            "tri": tri, "eye": eye, "ones": ones,
        })
    return in_maps


_NC_CACHE = {}


def _get_nc(T):
    if T not in _NC_CACHE:
        _NC_CACHE[T] = build(T)
    return _NC_CACHE[T]


def kernel(x, wq, bq, wk, bk, wv, bv, wp, bp, trace=False):
    x = np.asarray(x)
    B, T, C = x.shape
    in_maps = make_core_inputs(
        x[0], np.asarray(wq), np.asarray(bq), np.asarray(wk), np.asarray(bk),
        np.asarray(wv), np.asarray(bv), np.asarray(wp))
    nc = _get_nc(T)
    res = run_bass_kernel_spmd(nc, in_maps, core_ids=list(range(N_CORES)),
                               trace=trace)
    out = res.results[0]["partial"].copy()
    for c in range(1, N_CORES):
        out += res.results[c]["partial"]
    out += np.asarray(bp)[None, :]
    kernel.last_result = res
    return out[None].astype(np.float32)


# revision 22
# speedup vs baseline: 1.8070x; 1.2822x over previous
"""Causal self-attention (B=1, T=4096, C=1024, H=16) on 8 Trainium2 cores.

Sharding: tensor-parallel over heads. Each core owns 2 heads (128 channels):
  - computes q/k/v projections for its channels against the full sequence
    (x passed transposed as [C, T] so the contraction dim lands on partitions),
  - runs causal attention for its 2 heads in S^T = [k, q] layout
    (exp on ScalarE with fused 1/sqrt(D) scale; an appended ones-column in v
    yields softmax denominators as an extra output row of the same matmul),
  - multiplies its y slice against its 128 rows of wp -> a full [T, C] partial.
Host sums the 8 partials and adds the output bias. Identical SPMD program on
all cores; per-core data arrives via the input maps.

Dtype strategy (flags below): bf16 runs the PE at 1 cycle/row and keeps the
HAM clock-gate warm; float32r is 2 cycles/row but ~13-bit accurate. The
softmax normalizer path always stays float32r/fp32.
"""

import numpy as np
import ml_dtypes

import concourse.bass as bass
import concourse.mybir as mybir
import concourse.tile as tile
from concourse import bacc
from concourse.bass_utils import run_bass_kernel_spmd

P = 128
N_EMBD = 1024
N_HEAD = 16
T_FULL = 4096
N_CORES = 8
D = N_EMBD // N_HEAD          # 64
C_LOCAL = 2 * D               # 128 channels per core (2 heads)
F32 = mybir.dt.float32
F32R = mybir.dt.float32r
BF16 = mybir.dt.bfloat16
ADD = mybir.AluOpType.add
MULT = mybir.AluOpType.mult
EXP = mybir.ActivationFunctionType.Exp
LOG = mybir.ActivationFunctionType.Ln if hasattr(mybir.ActivationFunctionType, "Ln") else mybir.ActivationFunctionType.Log
Y_LAG = 3  # y-matmul trails S/exp by this many k-blocks

QT = 512   # attention q-tile (matmul moving free dim)
TS = 512   # projection t-chunk

# dtype switches
PROJ_BF16 = True   # x / qkv-weight operands
ATT_BF16 = True     # qT/kT (S^T matmul) and P/v (y matmul) operands
OUT_BF16 = True    # yT / wp operands


def _np_dt(dt):
    return ml_dtypes.bfloat16 if dt == BF16 else np.float32


def build(T=T_FULL):
    n_ts = T // TS
    n_qt = T // QT
    n_kb = T // P
    n_ct = N_EMBD // P
    kb_per_qt = QT // P

    XW_DT = BF16 if PROJ_BF16 else F32R
    QK_DT = BF16 if ATT_BF16 else F32R
    PV_DT = BF16 if ATT_BF16 else F32R
    Y_DT = BF16 if OUT_BF16 else F32R

    nc = bacc.Bacc("TRN2", target_bir_lowering=False, debug=False,
                   num_devices=N_CORES)
    xdt = BF16 if PROJ_BF16 else F32
    xT = nc.dram_tensor("xT", [N_EMBD, T], xdt, kind="ExternalInput")
    w3T = nc.dram_tensor("w3T", [N_EMBD, 3 * C_LOCAL], xdt,
                         kind="ExternalInput")
    b3 = nc.dram_tensor("b3", [C_LOCAL, 3], F32, kind="ExternalInput")
    wpdt = BF16 if OUT_BF16 else F32
    wpT = nc.dram_tensor("wpT", [C_LOCAL, N_EMBD], wpdt, kind="ExternalInput")
    tdt = BF16 if ATT_BF16 else F32
    tri = nc.dram_tensor("tri", [P, P], tdt, kind="ExternalInput")
    eye = nc.dram_tensor("eye", [P, P], tdt, kind="ExternalInput")
    ones = nc.dram_tensor("ones", [P, P], F32, kind="ExternalInput")
    sel2 = nc.dram_tensor("sel2", [P, P], F32, kind="ExternalInput")
    partial = nc.dram_tensor("partial", [T, N_EMBD], F32,
                             kind="ExternalOutput")

    def _in(ap, dt):
        return ap.bitcast(dt) if dt in (F32R,) else ap

    with tile.TileContext(nc) as tc:
        with (
            tc.tile_pool(name="const", bufs=1) as cpool,
            tc.tile_pool(name="psmall", bufs=2) as psmall,
            tc.tile_pool(name="pt", bufs=4) as ptp,
            tc.tile_pool(name="osb", bufs=2) as osbp,
        ):
            # ---- constants ----
            w3_sb = cpool.tile([P, n_ct, 3 * C_LOCAL], XW_DT, tag="w3")
            nc.sync.dma_start(
                w3_sb[:],
                _in(w3T.ap(), XW_DT).rearrange("(co p) o -> p co o", p=P))
            wp_sb = cpool.tile([P, N_EMBD], Y_DT, tag="wp")
            nc.sync.dma_start(wp_sb[:], _in(wpT.ap(), Y_DT))
            b3_sb = cpool.tile([P, 3], F32, tag="b3")
            nc.sync.dma_start(b3_sb[:], b3.ap())
            tri_sb = cpool.tile([P, P], PV_DT, tag="tri")
            nc.sync.dma_start(tri_sb[:], _in(tri.ap(), PV_DT))
            eye_sb = cpool.tile([P, P], PV_DT, tag="eye")
            nc.sync.dma_start(eye_sb[:], _in(eye.ap(), PV_DT))
            ones_sb = cpool.tile([P, P], F32R, tag="ones")
            nc.sync.dma_start(ones_sb[:], ones.ap().bitcast(F32R))
            ones1_sb = ones_sb[0:1, :]
            sel2_sb = cpool.tile([P, P], F32R, tag="sel2")
            nc.sync.dma_start(sel2_sb[:], sel2.ap().bitcast(F32R))
            s2_sb = cpool.tile([P, QT], F32R, tag="s2p")
            nc.vector.tensor_copy(
                s2_sb[:], ones_sb[:, 0:1].to_broadcast((P, QT)))

            qT_sb = cpool.tile([P, T], QK_DT, tag="qT")
            kT_sb = cpool.tile([P, T], QK_DT, tag="kT")
            yT_sb = cpool.tile([P, T], Y_DT, tag="yT")
            # v in [t-part, (head0 d + one | head1 d + one)] layout
            v_sb = cpool.tile([P, n_kb, 2 * (D + 1)], PV_DT, tag="v")
            nc.vector.tensor_copy(
                v_sb[:, :, D], ones_sb[:, 0:1].to_broadcast((P, n_kb)))
            nc.vector.tensor_copy(
                v_sb[:, :, 2 * D + 1], ones_sb[:, 0:1].to_broadcast((P, n_kb)))

            # ---- q/k/v projections ----
            with (
                tc.tile_pool(name="xin", bufs=3) as xin,
                tc.tile_pool(name="proj_ps", bufs=2, space="PSUM") as pps,
                tc.tile_pool(name="tr_ps", bufs=2, space="PSUM") as trps,
            ):
                XCH = min(2 * TS, T)  # 1024-wide chunks: 2 KiB rows (bf16)
                NHF = XCH // TS
                for tsp in range(T // XCH):
                    prj = [[pps.tile([P, TS], F32, tag=f"prj{j}",
                                     name=f"prj{j}_{tsp}_{hf}")
                            for j in range(3)] for hf in range(NHF)]
                    for c in range(n_ct):
                        x_t = xin.tile([P, XCH], XW_DT, tag="x")
                        nc.sync.dma_start(
                            x_t[:],
                            _in(xT.ap(), XW_DT)[c * P:(c + 1) * P,
                                                tsp * XCH:(tsp + 1) * XCH])
                        for hf in range(NHF):
                            for j in range(3):
                                nc.tensor.matmul(
                                    prj[hf][j][:],
                                    w3_sb[:, c, j * P:(j + 1) * P],
                                    x_t[:, hf * TS:(hf + 1) * TS],
                                    start=(c == 0), stop=(c == n_ct - 1))
                    for hf in range(NHF):
                        t0 = tsp * XCH + hf * TS
                        # bias add + copy out of PSUM
                        nc.vector.tensor_tensor(
                            qT_sb[:, t0:t0 + TS], prj[hf][0][:],
                            b3_sb[:, 0:1].to_broadcast((P, TS)), ADD)
                        nc.vector.tensor_tensor(
                            kT_sb[:, t0:t0 + TS], prj[hf][1][:],
                            b3_sb[:, 1:2].to_broadcast((P, TS)), ADD)
                        vt_t = xin.tile([P, TS], PV_DT, tag="vt")
                        nc.vector.tensor_tensor(
                            vt_t[:], prj[hf][2][:],
                            b3_sb[:, 2:3].to_broadcast((P, TS)), ADD)
                        # transpose v chunk into [t, c_local] layout
                        for kk in range(TS // P):
                            gkb = (t0 + kk * P) // P
                            t_ps = trps.tile([P, P], PV_DT, tag="tr")
                            nc.tensor.transpose(
                                t_ps[:], vt_t[:, kk * P:(kk + 1) * P],
                                eye_sb[:])
                            for h in range(2):
                                nc.vector.tensor_copy(
                                    v_sb[:, gkb, h * (D + 1):h * (D + 1) + D],
                                    t_ps[:, h * D:(h + 1) * D])

            # ---- attention + interleaved output projection ----
            with (
                tc.tile_pool(name="st_ps", bufs=2, space="PSUM") as stps,
                tc.tile_pool(name="y_ps", bufs=1, space="PSUM") as yps,
                tc.tile_pool(name="o_ps", bufs=1, space="PSUM") as ops,
            ):
                def emit_outproj(qt):
                    # output projection for the 4 t-blocks of q-tile qt
                    for tb in range(qt * (QT // P), (qt + 1) * (QT // P)):
                        o_ps = [ops.tile([P, 512], F32, tag=f"o{i}",
                                         name=f"o{i}_{tb}") for i in range(2)]
                        o_sb = osbp.tile([P, N_EMBD], F32, tag="osb")
                        for i in range(2):
                            nc.tensor.matmul(
                                o_ps[i][:],
                                yT_sb[:, tb * P:(tb + 1) * P],
                                wp_sb[:, i * 512:(i + 1) * 512],
                                start=True, stop=True)
                            nc.vector.tensor_copy(
                                o_sb[:, i * 512:(i + 1) * 512], o_ps[i][:])
                        nc.sync.dma_start(
                            partial.ap()[tb * P:(tb + 1) * P, :], o_sb[:])

                pending_norm = None
                pending_out = None

                def finish_norm(nqt, y_ps_n):
                    R_ps = ops.tile([P, QT], F32, tag="o0", name=f"R_{nqt}")
                    nc.tensor.matmul(R_ps[:], sel2_sb[:], s2_sb[:],
                                     start=True, stop=True)
                    R_sb = psmall.tile([P, QT], F32, tag="Rs")
                    nc.vector.reciprocal_approx_fast(R_sb[:], R_ps[:])
                    for h in range(2):
                        nc.vector.tensor_tensor(
                            yT_sb[D * h:D * (h + 1),
                                  nqt * QT:(nqt + 1) * QT],
                            y_ps_n[h][0:D, :],
                            R_sb[D * h:D * (h + 1), :],
                            MULT)

                for qt in range(n_qt):
                    q0 = qt * QT
                    nkb = (qt + 1) * kb_per_qt
                    y_ps = [yps.tile([P, QT], F32, tag=f"y{h}",
                                     name=f"y{h}_{qt}") for h in range(2)]

                    # software-pipelined: y-matmul trails S/exp by Y_LAG
                    from collections import deque
                    pend_y = deque()

                    def emit_y(kb, p_pair, first, last):
                        dg = kb >= qt * kb_per_qt
                        cc = (kb - qt * kb_per_qt) * P if dg else 0
                        for h in range(2):
                            nc.tensor.matmul(
                                y_ps[h][0:D + 1, cc:QT],
                                v_sb[:, kb, h * (D + 1):(h + 1) * (D + 1)],
                                p_pair[:, h, cc:QT],
                                start=first, stop=last)

                    for kb in range(nkb):
                        diag = kb >= qt * kb_per_qt
                        c = (kb - qt * kb_per_qt) * P if diag else 0
                        # both heads' scores in one 2-bank tile -> one exp
                        s_pair = stps.tile([P, 2, QT], F32, tag="s",
                                           name=f"s_{qt}_{kb}")
                        for h in range(2):
                            nc.tensor.matmul(
                                s_pair[:, h, c:QT],
                                kT_sb[D * h:D * h + D, kb * P:(kb + 1) * P],
                                qT_sb[D * h:D * h + D, q0 + c:q0 + QT],
                                start=True, stop=True)
                        p_pair = ptp.tile([P, 2, QT], PV_DT, tag="p",
                                          name=f"p_{qt}_{kb}")
                        nc.scalar.activation(
                            p_pair[:, :, c:QT], s_pair[:, :, c:QT], EXP,
                            scale=1.0 / float(np.sqrt(D)))
                        if diag:
                            for h in range(2):
                                nc.vector.tensor_tensor(
                                    p_pair[:, h, c:c + P], p_pair[:, h, c:c + P],
                                    tri_sb[:], MULT)
                        if kb == 1 and pending_norm is not None:
                            finish_norm(*pending_norm)
                            pending_norm = None
                        if kb == 2 and pending_out is not None:
                            emit_outproj(pending_out)
                            pending_out = None
                        pend_y.append((kb, p_pair))
                        if len(pend_y) > Y_LAG:
                            ykb, yp = pend_y.popleft()
                            emit_y(ykb, yp, ykb == 0, False)
                    while pend_y:
                        ykb, yp = pend_y.popleft()
                        emit_y(ykb, yp, ykb == 0, ykb == nkb - 1)

                    # softmax sums -> rows 0 / 64 of the persistent s2 tile;
                    # the selector-matmul broadcast + divide run deferred,
                    # inside the next q-tile's instruction stream.
                    for h in range(2):
                        nc.vector.tensor_copy(s2_sb[64 * h:64 * h + 1, :],
                                              y_ps[h][D:D + 1, :])
                    if pending_out is not None:
                        emit_outproj(pending_out)
                    pending_norm = (qt, y_ps)
                    pending_out = qt
                if pending_norm is not None:
                    finish_norm(*pending_norm)
                emit_outproj(pending_out)

    nc.compile()
    return nc


def make_core_inputs(x, wq, bq, wk, bk, wv, bv, wp):
    """Per-core input dicts (host-side sharding). x: [T, C] fp32."""
    xw_np = _np_dt(BF16 if PROJ_BF16 else F32)
    y_np = _np_dt(BF16 if OUT_BF16 else F32)
    t_np = _np_dt(BF16 if ATT_BF16 else F32)
    xT = np.ascontiguousarray(x.T).astype(xw_np)
    tri = (np.arange(P)[None, :] >= np.arange(P)[:, None]).astype(t_np)
    eye = np.eye(P, dtype=np.float32).astype(t_np)
    ones = np.ones((P, P), dtype=np.float32)
    sel2 = np.zeros((P, P), dtype=np.float32)
    sel2[0, :D] = 1.0
    sel2[64, D:2 * D] = 1.0
    in_maps = []
    for core in range(N_CORES):
        ch = slice(core * C_LOCAL, (core + 1) * C_LOCAL)
        w3T = np.ascontiguousarray(
            np.concatenate([wq[ch].T, wk[ch].T, wv[ch].T], axis=1)
        ).astype(xw_np)
        b3 = np.ascontiguousarray(
            np.stack([bq[ch], bk[ch], bv[ch]], axis=1)).astype(np.float32)
        wpT = np.ascontiguousarray(wp[:, ch].T).astype(y_np)
        in_maps.append({
            "xT": xT, "w3T": w3T, "b3": b3, "wpT": wpT,
            "tri": tri, "eye": eye, "ones": ones, "sel2": sel2,
        })
    return in_maps


_NC_CACHE = {}


def _get_nc(T):
    if T not in _NC_CACHE:
        _NC_CACHE[T] = build(T)
    return _NC_CACHE[T]


def kernel(x, wq, bq, wk, bk, wv, bv, wp, bp, trace=False):
    x = np.asarray(x)
    B, T, C = x.shape
    in_maps = make_core_inputs(
        x[0], np.asarray(wq), np.asarray(bq), np.asarray(wk), np.asarray(bk),
        np.asarray(wv), np.asarray(bv), np.asarray(wp))
    nc = _get_nc(T)
    res = run_bass_kernel_spmd(nc, in_maps, core_ids=list(range(N_CORES)),
                               trace=trace)
    out = res.results[0]["partial"].copy()
    for c in range(1, N_CORES):
        out += res.results[c]["partial"]
    out += np.asarray(bp)[None, :]
    kernel.last_result = res
    return out[None].astype(np.float32)


# revision 25
# speedup vs baseline: 1.9793x; 1.0953x over previous
"""Causal self-attention (B=1, T=4096, C=1024, H=16) on 8 Trainium2 cores.

Sharding: tensor-parallel over heads. Each core owns 2 heads (128 channels):
  - computes q/k/v projections for its channels against the full sequence
    (x passed transposed as [C, T] so the contraction dim lands on partitions),
  - runs causal attention for its 2 heads in S^T = [k, q] layout
    (exp on ScalarE with fused 1/sqrt(D) scale; an appended ones-column in v
    yields softmax denominators as an extra output row of the same matmul),
  - multiplies its y slice against its 128 rows of wp -> a full [T, C] partial.
Host sums the 8 partials and adds the output bias. Identical SPMD program on
all cores; per-core data arrives via the input maps.

Dtype strategy (flags below): bf16 runs the PE at 1 cycle/row and keeps the
HAM clock-gate warm; float32r is 2 cycles/row but ~13-bit accurate. The
softmax normalizer path always stays float32r/fp32.
"""

import numpy as np
import ml_dtypes

import concourse.bass as bass
import concourse.mybir as mybir
import concourse.tile as tile
from concourse import bacc
from concourse.bass_utils import run_bass_kernel_spmd

P = 128
N_EMBD = 1024
N_HEAD = 16
T_FULL = 4096
N_CORES = 8
D = N_EMBD // N_HEAD          # 64
C_LOCAL = 2 * D               # 128 channels per core (2 heads)
F32 = mybir.dt.float32
F32R = mybir.dt.float32r
BF16 = mybir.dt.bfloat16
ADD = mybir.AluOpType.add
MULT = mybir.AluOpType.mult
EXP = mybir.ActivationFunctionType.Exp
LOG = mybir.ActivationFunctionType.Ln if hasattr(mybir.ActivationFunctionType, "Ln") else mybir.ActivationFunctionType.Log
Y_LAG = 3  # y-matmul trails S/exp by this many k-blocks

QT = 512   # attention q-tile (matmul moving free dim)
TS = 512   # projection t-chunk

# dtype switches
PROJ_BF16 = True   # x / qkv-weight operands
ATT_BF16 = True     # qT/kT (S^T matmul) and P/v (y matmul) operands
OUT_BF16 = True    # yT / wp operands


def _np_dt(dt):
    return ml_dtypes.bfloat16 if dt == BF16 else np.float32


def build(T=T_FULL):
    n_ts = T // TS
    n_qt = T // QT
    n_kb = T // P
    n_ct = N_EMBD // P
    kb_per_qt = QT // P

    XW_DT = BF16 if PROJ_BF16 else F32R
    QK_DT = BF16 if ATT_BF16 else F32R
    PV_DT = BF16 if ATT_BF16 else F32R
    Y_DT = BF16 if OUT_BF16 else F32R

    nc = bacc.Bacc("TRN2", target_bir_lowering=False, debug=False,
                   num_devices=N_CORES)
    xdt = BF16 if PROJ_BF16 else F32
    xT = nc.dram_tensor("xT", [N_EMBD, T], xdt, kind="ExternalInput")
    w3T = nc.dram_tensor("w3T", [N_EMBD, 3 * C_LOCAL], xdt,
                         kind="ExternalInput")
    b3 = nc.dram_tensor("b3", [C_LOCAL, 3], F32, kind="ExternalInput")
    wpdt = BF16 if OUT_BF16 else F32
    wpT = nc.dram_tensor("wpT", [C_LOCAL, N_EMBD], wpdt, kind="ExternalInput")
    tdt = BF16 if ATT_BF16 else F32
    tri = nc.dram_tensor("tri", [P, P], tdt, kind="ExternalInput")
    eye = nc.dram_tensor("eye", [P, P], tdt, kind="ExternalInput")
    ones = nc.dram_tensor("ones", [P, P], F32, kind="ExternalInput")
    sel2 = nc.dram_tensor("sel2", [P, P], F32, kind="ExternalInput")
    partial = nc.dram_tensor("partial", [T, N_EMBD], F32,
                             kind="ExternalOutput")

    def _in(ap, dt):
        return ap.bitcast(dt) if dt in (F32R,) else ap

    with tile.TileContext(nc) as tc:
        with (
            tc.tile_pool(name="const", bufs=1) as cpool,
            tc.tile_pool(name="psmall", bufs=2) as psmall,
            tc.tile_pool(name="pt", bufs=4) as ptp,
            tc.tile_pool(name="osb", bufs=2) as osbp,
        ):
            # ---- constants ----
            w3_sb = cpool.tile([P, n_ct, 3 * C_LOCAL], XW_DT, tag="w3")
            nc.sync.dma_start(
                w3_sb[:],
                _in(w3T.ap(), XW_DT).rearrange("(co p) o -> p co o", p=P))
            wp_sb = cpool.tile([P, N_EMBD], Y_DT, tag="wp")
            nc.sync.dma_start(wp_sb[:], _in(wpT.ap(), Y_DT))
            b3_sb = cpool.tile([P, 3], F32, tag="b3")
            nc.sync.dma_start(b3_sb[:], b3.ap())
            tri_sb = cpool.tile([P, P], PV_DT, tag="tri")
            nc.sync.dma_start(tri_sb[:], _in(tri.ap(), PV_DT))
            eye_sb = cpool.tile([P, P], PV_DT, tag="eye")
            nc.sync.dma_start(eye_sb[:], _in(eye.ap(), PV_DT))
            ones_sb = cpool.tile([P, P], F32R, tag="ones")
            nc.sync.dma_start(ones_sb[:], ones.ap().bitcast(F32R))
            ones1_sb = ones_sb[0:1, :]
            sel2_sb = cpool.tile([P, P], F32R, tag="sel2")
            nc.sync.dma_start(sel2_sb[:], sel2.ap().bitcast(F32R))
            s2_sb = cpool.tile([P, QT], F32R, tag="s2p")
            nc.vector.tensor_copy(
                s2_sb[:], ones_sb[:, 0:1].to_broadcast((P, QT)))

            qT_sb = cpool.tile([P, T], QK_DT, tag="qT")
            kT_sb = cpool.tile([P, T], QK_DT, tag="kT")
            yT_sb = cpool.tile([P, T], Y_DT, tag="yT")
            # v in [t-part, (head0 d + one | head1 d + one)] layout
            v_sb = cpool.tile([P, n_kb, 2 * (D + 1)], PV_DT, tag="v")
            nc.vector.tensor_copy(
                v_sb[:, :, D], ones_sb[:, 0:1].to_broadcast((P, n_kb)))
            nc.vector.tensor_copy(
                v_sb[:, :, 2 * D + 1], ones_sb[:, 0:1].to_broadcast((P, n_kb)))

            # ---- q/k/v projections ----
            # x is fully resident in SBUF, fetched as whole rows (8 KiB
            # contiguous per partition) in two T-halves so the first
            # t-chunks' matmuls can start while the second half streams in.
            xfull = cpool.tile([P, n_ct, T], XW_DT, tag="xfull")
            T2 = T // 2
            for hlf in range(2):
                for c in range(n_ct):
                    nc.sync.dma_start(
                        xfull[:, c, hlf * T2:(hlf + 1) * T2],
                        _in(xT.ap(), XW_DT)[c * P:(c + 1) * P,
                                            hlf * T2:(hlf + 1) * T2])
            with (
                tc.tile_pool(name="xin", bufs=3) as xin,
                tc.tile_pool(name="proj_ps", bufs=2, space="PSUM") as pps,
                tc.tile_pool(name="tr_ps", bufs=2, space="PSUM") as trps,
            ):
                XCH = min(2 * TS, T)
                NHF = XCH // TS
                for tsp in range(T // XCH):
                    prj = [[pps.tile([P, TS], F32, tag=f"prj{j}",
                                     name=f"prj{j}_{tsp}_{hf}")
                            for j in range(3)] for hf in range(NHF)]
                    for c in range(n_ct):
                        for hf in range(NHF):
                            for j in range(3):
                                nc.tensor.matmul(
                                    prj[hf][j][:],
                                    w3_sb[:, c, j * P:(j + 1) * P],
                                    xfull[:, c, tsp * XCH + hf * TS:
                                          tsp * XCH + (hf + 1) * TS],
                                    start=(c == 0), stop=(c == n_ct - 1))
                    for hf in range(NHF):
                        t0 = tsp * XCH + hf * TS
                        # bias add + copy out of PSUM
                        nc.vector.tensor_tensor(
                            qT_sb[:, t0:t0 + TS], prj[hf][0][:],
                            b3_sb[:, 0:1].to_broadcast((P, TS)), ADD)
                        nc.vector.tensor_tensor(
                            kT_sb[:, t0:t0 + TS], prj[hf][1][:],
                            b3_sb[:, 1:2].to_broadcast((P, TS)), ADD)
                        vt_t = xin.tile([P, TS], PV_DT, tag="vt")
                        nc.vector.tensor_tensor(
                            vt_t[:], prj[hf][2][:],
                            b3_sb[:, 2:3].to_broadcast((P, TS)), ADD)
                        # transpose v chunk into [t, c_local] layout
                        for kk in range(TS // P):
                            gkb = (t0 + kk * P) // P
                            t_ps = trps.tile([P, P], PV_DT, tag="tr")
                            nc.tensor.transpose(
                                t_ps[:], vt_t[:, kk * P:(kk + 1) * P],
                                eye_sb[:])
                            for h in range(2):
                                nc.vector.tensor_copy(
                                    v_sb[:, gkb, h * (D + 1):h * (D + 1) + D],
                                    t_ps[:, h * D:(h + 1) * D])

            # ---- attention + interleaved output projection ----
            with (
                tc.tile_pool(name="st_ps", bufs=2, space="PSUM") as stps,
                tc.tile_pool(name="y_ps", bufs=1, space="PSUM") as yps,
                tc.tile_pool(name="o_ps", bufs=1, space="PSUM") as ops,
            ):
                def emit_outproj(qt):
                    # output projection for the 4 t-blocks of q-tile qt
                    for tb in range(qt * (QT // P), (qt + 1) * (QT // P)):
                        o_ps = [ops.tile([P, 512], F32, tag=f"o{i}",
                                         name=f"o{i}_{tb}") for i in range(2)]
                        o_sb = osbp.tile([P, N_EMBD], F32, tag="osb")
                        for i in range(2):
                            nc.tensor.matmul(
                                o_ps[i][:],
                                yT_sb[:, tb * P:(tb + 1) * P],
                                wp_sb[:, i * 512:(i + 1) * 512],
                                start=True, stop=True)
                            nc.vector.tensor_copy(
                                o_sb[:, i * 512:(i + 1) * 512], o_ps[i][:])
                        nc.sync.dma_start(
                            partial.ap()[tb * P:(tb + 1) * P, :], o_sb[:])

                from collections import deque
                pend_y = deque()
                pending_norm = None
                pending_out = None
                pending_tail = None

                def finish_norm(nqt, y_ps_n):
                    R_ps = ops.tile([P, QT], F32, tag="o0", name=f"R_{nqt}")
                    nc.tensor.matmul(R_ps[:], sel2_sb[:], s2_sb[:],
                                     start=True, stop=True)
                    R_sb = psmall.tile([P, QT], F32, tag="Rs")
                    nc.vector.reciprocal_approx_fast(R_sb[:], R_ps[:])
                    for h in range(2):
                        nc.vector.tensor_tensor(
                            yT_sb[D * h:D * (h + 1),
                                  nqt * QT:(nqt + 1) * QT],
                            y_ps_n[h][0:D, :],
                            R_sb[D * h:D * (h + 1), :],
                            MULT)

                for qt in range(n_qt):
                    q0 = qt * QT
                    nkb = (qt + 1) * kb_per_qt
                    y_ps = [yps.tile([P, QT], F32, tag=f"y{h}",
                                     name=f"y{h}_{qt}") for h in range(2)]

                    # software-pipelined: y-matmul trails S/exp by Y_LAG;
                    # the tail of each q-tile (last y-matmuls + sum copies)
                    # is deferred into the next q-tile's stream.
                    def emit_y(kb, p_pair, y_ps_l, nkb_l, q_first):
                        dg = kb >= q_first
                        cc = (kb - q_first) * P if dg else 0
                        for h in range(2):
                            nc.tensor.matmul(
                                y_ps_l[h][0:D + 1, cc:QT],
                                v_sb[:, kb, h * (D + 1):(h + 1) * (D + 1)],
                                p_pair[:, h, cc:QT],
                                start=(kb == 0), stop=(kb == nkb_l - 1))

                    for kb in range(nkb):
                        diag = kb >= qt * kb_per_qt
                        c = (kb - qt * kb_per_qt) * P if diag else 0
                        # both heads' scores in one 2-bank tile -> one exp
                        s_pair = stps.tile([P, 2, QT], F32, tag="s",
                                           name=f"s_{qt}_{kb}")
                        for h in range(2):
                            nc.tensor.matmul(
                                s_pair[:, h, c:QT],
                                kT_sb[D * h:D * h + D, kb * P:(kb + 1) * P],
                                qT_sb[D * h:D * h + D, q0 + c:q0 + QT],
                                start=True, stop=True)
                        p_pair = ptp.tile([P, 2, QT], PV_DT, tag="p",
                                          name=f"p_{qt}_{kb}")
                        nc.scalar.activation(
                            p_pair[:, :, c:QT], s_pair[:, :, c:QT], EXP,
                            scale=1.0 / float(np.sqrt(D)))
                        if diag:
                            for h in range(2):
                                nc.vector.tensor_tensor(
                                    p_pair[:, h, c:c + P],
                                    p_pair[:, h, c:c + P],
                                    tri_sb[:], MULT)
                        if kb == 0 and pending_tail is not None:
                            pending_tail()
                            pending_tail = None
                        if kb == 1 and pending_norm is not None:
                            finish_norm(*pending_norm)
                            pending_norm = None
                        if kb == 2 and pending_out is not None:
                            emit_outproj(pending_out)
                            pending_out = None
                        pend_y.append(
                            (lambda a, b, c_, d_, e_:
                             lambda: emit_y(a, b, c_, d_, e_))(
                                 kb, p_pair, y_ps, nkb, qt * kb_per_qt))
                        while len(pend_y) > Y_LAG:
                            pend_y.popleft()()

                    def make_tail(y_ps_t, ndrain):
                        def _tail():
                            for _ in range(ndrain):
                                pend_y.popleft()()
                            for h in range(2):
                                nc.vector.tensor_copy(
                                    s2_sb[64 * h:64 * h + 1, :],
                                    y_ps_t[h][D:D + 1, :])
                        return _tail

                    pending_tail = make_tail(y_ps, len(pend_y))
                    if pending_out is not None:
                        emit_outproj(pending_out)
                        pending_out = None
                    pending_norm = (qt, y_ps)
                    pending_out = qt
                if pending_tail is not None:
                    pending_tail()
                if pending_norm is not None:
                    finish_norm(*pending_norm)
                emit_outproj(pending_out)

    nc.compile()
    return nc


def make_core_inputs(x, wq, bq, wk, bk, wv, bv, wp):
    """Per-core input dicts (host-side sharding). x: [T, C] fp32."""
    xw_np = _np_dt(BF16 if PROJ_BF16 else F32)
    y_np = _np_dt(BF16 if OUT_BF16 else F32)
    t_np = _np_dt(BF16 if ATT_BF16 else F32)
    xT = np.ascontiguousarray(x.T).astype(xw_np)
    tri = (np.arange(P)[None, :] >= np.arange(P)[:, None]).astype(t_np)
    eye = np.eye(P, dtype=np.float32).astype(t_np)
    ones = np.ones((P, P), dtype=np.float32)
    sel2 = np.zeros((P, P), dtype=np.float32)
    sel2[0, :D] = 1.0
    sel2[64, D:2 * D] = 1.0
    in_maps = []
    for core in range(N_CORES):
        ch = slice(core * C_LOCAL, (core + 1) * C_LOCAL)
        w3T = np.ascontiguousarray(
            np.concatenate([wq[ch].T, wk[ch].T, wv[ch].T], axis=1)
        ).astype(xw_np)
        b3 = np.ascontiguousarray(
            np.stack([bq[ch], bk[ch], bv[ch]], axis=1)).astype(np.float32)
        wpT = np.ascontiguousarray(wp[:, ch].T).astype(y_np)
        in_maps.append({
            "xT": xT, "w3T": w3T, "b3": b3, "wpT": wpT,
            "tri": tri, "eye": eye, "ones": ones, "sel2": sel2,
        })
    return in_maps


_NC_CACHE = {}


def _get_nc(T):
    if T not in _NC_CACHE:
        _NC_CACHE[T] = build(T)
    return _NC_CACHE[T]


def kernel(x, wq, bq, wk, bk, wv, bv, wp, bp, trace=False):
    x = np.asarray(x)
    B, T, C = x.shape
    in_maps = make_core_inputs(
        x[0], np.asarray(wq), np.asarray(bq), np.asarray(wk), np.asarray(bk),
        np.asarray(wv), np.asarray(bv), np.asarray(wp))
    nc = _get_nc(T)
    res = run_bass_kernel_spmd(nc, in_maps, core_ids=list(range(N_CORES)),
                               trace=trace)
    out = res.results[0]["partial"].copy()
    for c in range(1, N_CORES):
        out += res.results[c]["partial"]
    out += np.asarray(bp)[None, :]
    kernel.last_result = res
    return out[None].astype(np.float32)


# revision 26
# speedup vs baseline: 2.0397x; 1.0305x over previous
"""Causal self-attention (B=1, T=4096, C=1024, H=16) on 8 Trainium2 cores.

Sharding: tensor-parallel over heads. Each core owns 2 heads (128 channels):
  - computes q/k/v projections for its channels against the full sequence
    (x passed transposed as [C, T] so the contraction dim lands on partitions),
  - runs causal attention for its 2 heads in S^T = [k, q] layout
    (exp on ScalarE with fused 1/sqrt(D) scale; an appended ones-column in v
    yields softmax denominators as an extra output row of the same matmul),
  - multiplies its y slice against its 128 rows of wp -> a full [T, C] partial.
Host sums the 8 partials and adds the output bias. Identical SPMD program on
all cores; per-core data arrives via the input maps.

Dtype strategy (flags below): bf16 runs the PE at 1 cycle/row and keeps the
HAM clock-gate warm; float32r is 2 cycles/row but ~13-bit accurate. The
softmax normalizer path always stays float32r/fp32.
"""

import numpy as np
import ml_dtypes

import concourse.bass as bass
import concourse.mybir as mybir
import concourse.tile as tile
from concourse import bacc
from concourse.bass_utils import run_bass_kernel_spmd

P = 128
N_EMBD = 1024
N_HEAD = 16
T_FULL = 4096
N_CORES = 8
D = N_EMBD // N_HEAD          # 64
C_LOCAL = 2 * D               # 128 channels per core (2 heads)
F32 = mybir.dt.float32
F32R = mybir.dt.float32r
BF16 = mybir.dt.bfloat16
ADD = mybir.AluOpType.add
MULT = mybir.AluOpType.mult
EXP = mybir.ActivationFunctionType.Exp
LOG = mybir.ActivationFunctionType.Ln if hasattr(mybir.ActivationFunctionType, "Ln") else mybir.ActivationFunctionType.Log
Y_LAG = 4  # y-matmul trails S/exp by this many k-blocks

QT = 512   # attention q-tile (matmul moving free dim)
TS = 512   # projection t-chunk

# dtype switches
PROJ_BF16 = True   # x / qkv-weight operands
ATT_BF16 = True     # qT/kT (S^T matmul) and P/v (y matmul) operands
OUT_BF16 = True    # yT / wp operands


def _np_dt(dt):
    return ml_dtypes.bfloat16 if dt == BF16 else np.float32


def build(T=T_FULL):
    n_ts = T // TS
    n_qt = T // QT
    n_kb = T // P
    n_ct = N_EMBD // P
    kb_per_qt = QT // P

    XW_DT = BF16 if PROJ_BF16 else F32R
    QK_DT = BF16 if ATT_BF16 else F32R
    PV_DT = BF16 if ATT_BF16 else F32R
    Y_DT = BF16 if OUT_BF16 else F32R

    nc = bacc.Bacc("TRN2", target_bir_lowering=False, debug=False,
                   num_devices=N_CORES)
    xdt = BF16 if PROJ_BF16 else F32
    xT = nc.dram_tensor("xT", [N_EMBD, T], xdt, kind="ExternalInput")
    w3T = nc.dram_tensor("w3T", [N_EMBD, 3 * C_LOCAL], xdt,
                         kind="ExternalInput")
    b3 = nc.dram_tensor("b3", [C_LOCAL, 3], F32, kind="ExternalInput")
    wpdt = BF16 if OUT_BF16 else F32
    wpT = nc.dram_tensor("wpT", [C_LOCAL, N_EMBD], wpdt, kind="ExternalInput")
    tdt = BF16 if ATT_BF16 else F32
    tri = nc.dram_tensor("tri", [P, P], tdt, kind="ExternalInput")
    eye = nc.dram_tensor("eye", [P, P], tdt, kind="ExternalInput")
    ones = nc.dram_tensor("ones", [P, P], F32, kind="ExternalInput")
    sel2 = nc.dram_tensor("sel2", [P, P], F32, kind="ExternalInput")
    partial = nc.dram_tensor("partial", [T, N_EMBD], F32,
                             kind="ExternalOutput")

    def _in(ap, dt):
        return ap.bitcast(dt) if dt in (F32R,) else ap

    with tile.TileContext(nc) as tc:
        with (
            tc.tile_pool(name="const", bufs=1) as cpool,
            tc.tile_pool(name="psmall", bufs=2) as psmall,
            tc.tile_pool(name="pt", bufs=6) as ptp,
            tc.tile_pool(name="osb", bufs=2) as osbp,
        ):
            # ---- constants ----
            w3_sb = cpool.tile([P, n_ct, 3 * C_LOCAL], XW_DT, tag="w3")
            nc.sync.dma_start(
                w3_sb[:],
                _in(w3T.ap(), XW_DT).rearrange("(co p) o -> p co o", p=P))
            wp_sb = cpool.tile([P, N_EMBD], Y_DT, tag="wp")
            nc.sync.dma_start(wp_sb[:], _in(wpT.ap(), Y_DT))
            b3_sb = cpool.tile([P, 3], F32, tag="b3")
            nc.sync.dma_start(b3_sb[:], b3.ap())
            tri_sb = cpool.tile([P, P], PV_DT, tag="tri")
            nc.sync.dma_start(tri_sb[:], _in(tri.ap(), PV_DT))
            eye_sb = cpool.tile([P, P], PV_DT, tag="eye")
            nc.sync.dma_start(eye_sb[:], _in(eye.ap(), PV_DT))
            ones_sb = cpool.tile([P, P], F32R, tag="ones")
            nc.sync.dma_start(ones_sb[:], ones.ap().bitcast(F32R))
            ones1_sb = ones_sb[0:1, :]
            sel2_sb = cpool.tile([P, P], F32R, tag="sel2")
            nc.sync.dma_start(sel2_sb[:], sel2.ap().bitcast(F32R))
            s2_sb = cpool.tile([P, QT], F32R, tag="s2p")
            nc.vector.tensor_copy(
                s2_sb[:], ones_sb[:, 0:1].to_broadcast((P, QT)))

            qT_sb = cpool.tile([P, T], QK_DT, tag="qT")
            kT_sb = cpool.tile([P, T], QK_DT, tag="kT")
            yT_sb = cpool.tile([P, T], Y_DT, tag="yT")
            # v in [t-part, (head0 d + one | head1 d + one)] layout
            v_sb = cpool.tile([P, n_kb, 2 * (D + 1)], PV_DT, tag="v")
            nc.vector.tensor_copy(
                v_sb[:, :, D], ones_sb[:, 0:1].to_broadcast((P, n_kb)))
            nc.vector.tensor_copy(
                v_sb[:, :, 2 * D + 1], ones_sb[:, 0:1].to_broadcast((P, n_kb)))

            # ---- q/k/v projections ----
            # x is fully resident in SBUF, fetched as whole rows (8 KiB
            # contiguous per partition) in two T-halves so the first
            # t-chunks' matmuls can start while the second half streams in.
            xfull = cpool.tile([P, n_ct, T], XW_DT, tag="xfull")
            T4 = T // 4
            for qtr in range(4):
                for c in range(n_ct):
                    nc.sync.dma_start(
                        xfull[:, c, qtr * T4:(qtr + 1) * T4],
                        _in(xT.ap(), XW_DT)[c * P:(c + 1) * P,
                                            qtr * T4:(qtr + 1) * T4])
            with (
                tc.tile_pool(name="xin", bufs=3) as xin,
                tc.tile_pool(name="proj_ps", bufs=2, space="PSUM") as pps,
                tc.tile_pool(name="tr_ps", bufs=2, space="PSUM") as trps,
            ):
                XCH = min(2 * TS, T)
                NHF = XCH // TS
                for tsp in range(T // XCH):
                    prj = [[pps.tile([P, TS], F32, tag=f"prj{j}",
                                     name=f"prj{j}_{tsp}_{hf}")
                            for j in range(3)] for hf in range(NHF)]
                    for c in range(n_ct):
                        for hf in range(NHF):
                            for j in range(3):
                                nc.tensor.matmul(
                                    prj[hf][j][:],
                                    w3_sb[:, c, j * P:(j + 1) * P],
                                    xfull[:, c, tsp * XCH + hf * TS:
                                          tsp * XCH + (hf + 1) * TS],
                                    start=(c == 0), stop=(c == n_ct - 1))
                    for hf in range(NHF):
                        t0 = tsp * XCH + hf * TS
                        # bias add + copy out of PSUM
                        nc.vector.tensor_tensor(
                            qT_sb[:, t0:t0 + TS], prj[hf][0][:],
                            b3_sb[:, 0:1].to_broadcast((P, TS)), ADD)
                        nc.vector.tensor_tensor(
                            kT_sb[:, t0:t0 + TS], prj[hf][1][:],
                            b3_sb[:, 1:2].to_broadcast((P, TS)), ADD)
                        vt_t = xin.tile([P, TS], PV_DT, tag="vt")
                        nc.vector.tensor_tensor(
                            vt_t[:], prj[hf][2][:],
                            b3_sb[:, 2:3].to_broadcast((P, TS)), ADD)
                        # transpose v chunk into [t, c_local] layout
                        for kk in range(TS // P):
                            gkb = (t0 + kk * P) // P
                            t_ps = trps.tile([P, P], PV_DT, tag="tr")
                            nc.tensor.transpose(
                                t_ps[:], vt_t[:, kk * P:(kk + 1) * P],
                                eye_sb[:])
                            for h in range(2):
                                nc.vector.tensor_copy(
                                    v_sb[:, gkb, h * (D + 1):h * (D + 1) + D],
                                    t_ps[:, h * D:(h + 1) * D])

            # ---- attention + interleaved output projection ----
            with (
                tc.tile_pool(name="st_ps", bufs=2, space="PSUM") as stps,
                tc.tile_pool(name="y_ps", bufs=1, space="PSUM") as yps,
                tc.tile_pool(name="o_ps", bufs=1, space="PSUM") as ops,
            ):
                def emit_outproj(qt):
                    # output projection for the 4 t-blocks of q-tile qt
                    for tb in range(qt * (QT // P), (qt + 1) * (QT // P)):
                        o_ps = [ops.tile([P, 512], F32, tag=f"o{i}",
                                         name=f"o{i}_{tb}") for i in range(2)]
                        o_sb = osbp.tile([P, N_EMBD], F32, tag="osb")
                        for i in range(2):
                            nc.tensor.matmul(
                                o_ps[i][:],
                                yT_sb[:, tb * P:(tb + 1) * P],
                                wp_sb[:, i * 512:(i + 1) * 512],
                                start=True, stop=True)
                            nc.vector.tensor_copy(
                                o_sb[:, i * 512:(i + 1) * 512], o_ps[i][:])
                        nc.sync.dma_start(
                            partial.ap()[tb * P:(tb + 1) * P, :], o_sb[:])

                from collections import deque
                pend_y = deque()
                pending_norm = None
                pending_out = None
                pending_tail = None

                def finish_norm(nqt, y_ps_n):
                    R_ps = ops.tile([P, QT], F32, tag="o0", name=f"R_{nqt}")
                    nc.tensor.matmul(R_ps[:], sel2_sb[:], s2_sb[:],
                                     start=True, stop=True)
                    R_sb = psmall.tile([P, QT], F32, tag="Rs")
                    nc.vector.reciprocal_approx_fast(R_sb[:], R_ps[:])
                    for h in range(2):
                        nc.vector.tensor_tensor(
                            yT_sb[D * h:D * (h + 1),
                                  nqt * QT:(nqt + 1) * QT],
                            y_ps_n[h][0:D, :],
                            R_sb[D * h:D * (h + 1), :],
                            MULT)

                for qt in range(n_qt):
                    q0 = qt * QT
                    nkb = (qt + 1) * kb_per_qt
                    y_ps = [yps.tile([P, QT], F32, tag=f"y{h}",
                                     name=f"y{h}_{qt}") for h in range(2)]

                    # software-pipelined: y-matmul trails S/exp by Y_LAG;
                    # the tail of each q-tile (last y-matmuls + sum copies)
                    # is deferred into the next q-tile's stream.
                    def emit_y(kb, p_pair, y_ps_l, nkb_l, q_first):
                        dg = kb >= q_first
                        cc = (kb - q_first) * P if dg else 0
                        for h in range(2):
                            nc.tensor.matmul(
                                y_ps_l[h][0:D + 1, cc:QT],
                                v_sb[:, kb, h * (D + 1):(h + 1) * (D + 1)],
                                p_pair[:, h * QT + cc:(h + 1) * QT],
                                start=(kb == 0), stop=(kb == nkb_l - 1))

                    for kb in range(nkb):
                        diag = kb >= qt * kb_per_qt
                        c = (kb - qt * kb_per_qt) * P if diag else 0
                        # both heads' scores in one 2-bank tile -> one exp
                        s_pair = stps.tile([P, 2 * QT], F32, tag="s",
                                           name=f"s_{qt}_{kb}")
                        for h in range(2):
                            nc.tensor.matmul(
                                s_pair[:, h * QT + c:(h + 1) * QT],
                                kT_sb[D * h:D * h + D, kb * P:(kb + 1) * P],
                                qT_sb[D * h:D * h + D, q0 + c:q0 + QT],
                                start=True, stop=True)
                        p_pair = ptp.tile([P, 2 * QT], PV_DT, tag="p",
                                          name=f"p_{qt}_{kb}")
                        if diag:
                            nc.scalar.activation(
                                p_pair[:].rearrange(
                                    "p (h q) -> p h q", h=2)[:, :, c:QT],
                                s_pair[:].rearrange(
                                    "p (h q) -> p h q", h=2)[:, :, c:QT],
                                EXP, scale=1.0 / float(np.sqrt(D)))
                            for h in range(2):
                                nc.vector.tensor_tensor(
                                    p_pair[:, h * QT + c:h * QT + c + P],
                                    p_pair[:, h * QT + c:h * QT + c + P],
                                    tri_sb[:], MULT)
                        else:
                            nc.scalar.activation(
                                p_pair[:], s_pair[:], EXP,
                                scale=1.0 / float(np.sqrt(D)))
                        if kb == 0 and pending_tail is not None:
                            pending_tail()
                            pending_tail = None
                        if kb == 1 and pending_norm is not None:
                            finish_norm(*pending_norm)
                            pending_norm = None
                        if kb == 2 and pending_out is not None:
                            emit_outproj(pending_out)
                            pending_out = None
                        pend_y.append(
                            (lambda a, b, c_, d_, e_:
                             lambda: emit_y(a, b, c_, d_, e_))(
                                 kb, p_pair, y_ps, nkb, qt * kb_per_qt))
                        while len(pend_y) > Y_LAG:
                            pend_y.popleft()()

                    def make_tail(y_ps_t, ndrain):
                        def _tail():
                            for _ in range(ndrain):
                                pend_y.popleft()()
                            for h in range(2):
                                nc.vector.tensor_copy(
                                    s2_sb[64 * h:64 * h + 1, :],
                                    y_ps_t[h][D:D + 1, :])
                        return _tail

                    pending_tail = make_tail(y_ps, len(pend_y))
                    if pending_out is not None:
                        emit_outproj(pending_out)
                        pending_out = None
                    pending_norm = (qt, y_ps)
                    pending_out = qt
                if pending_tail is not None:
                    pending_tail()
                if pending_norm is not None:
                    finish_norm(*pending_norm)
                emit_outproj(pending_out)

    nc.compile()
    return nc


def make_core_inputs(x, wq, bq, wk, bk, wv, bv, wp):
    """Per-core input dicts (host-side sharding). x: [T, C] fp32."""
    xw_np = _np_dt(BF16 if PROJ_BF16 else F32)
    y_np = _np_dt(BF16 if OUT_BF16 else F32)
    t_np = _np_dt(BF16 if ATT_BF16 else F32)
    xT = np.ascontiguousarray(x.T).astype(xw_np)
    tri = (np.arange(P)[None, :] >= np.arange(P)[:, None]).astype(t_np)
    eye = np.eye(P, dtype=np.float32).astype(t_np)
    ones = np.ones((P, P), dtype=np.float32)
    sel2 = np.zeros((P, P), dtype=np.float32)
    sel2[0, :D] = 1.0
    sel2[64, D:2 * D] = 1.0
    in_maps = []
    for core in range(N_CORES):
        ch = slice(core * C_LOCAL, (core + 1) * C_LOCAL)
        w3T = np.ascontiguousarray(
            np.concatenate([wq[ch].T, wk[ch].T, wv[ch].T], axis=1)
        ).astype(xw_np)
        b3 = np.ascontiguousarray(
            np.stack([bq[ch], bk[ch], bv[ch]], axis=1)).astype(np.float32)
        wpT = np.ascontiguousarray(wp[:, ch].T).astype(y_np)
        in_maps.append({
            "xT": xT, "w3T": w3T, "b3": b3, "wpT": wpT,
            "tri": tri, "eye": eye, "ones": ones, "sel2": sel2,
        })
    return in_maps


_NC_CACHE = {}


def _get_nc(T):
    if T not in _NC_CACHE:
        _NC_CACHE[T] = build(T)
    return _NC_CACHE[T]


def kernel(x, wq, bq, wk, bk, wv, bv, wp, bp, trace=False):
    x = np.asarray(x)
    B, T, C = x.shape
    in_maps = make_core_inputs(
        x[0], np.asarray(wq), np.asarray(bq), np.asarray(wk), np.asarray(bk),
        np.asarray(wv), np.asarray(bv), np.asarray(wp))
    nc = _get_nc(T)
    res = run_bass_kernel_spmd(nc, in_maps, core_ids=list(range(N_CORES)),
                               trace=trace)
    out = res.results[0]["partial"].copy()
    for c in range(1, N_CORES):
        out += res.results[c]["partial"]
    out += np.asarray(bp)[None, :]
    kernel.last_result = res
    return out[None].astype(np.float32)


# revision 27
# speedup vs baseline: 2.0435x; 1.0019x over previous
"""Causal self-attention (B=1, T=4096, C=1024, H=16) on 8 Trainium2 cores.

Sharding: tensor-parallel over heads. Each core owns 2 heads (128 channels):
  - computes q/k/v projections for its channels against the full sequence
    (x passed transposed as [C, T] so the contraction dim lands on partitions),
  - runs causal attention for its 2 heads in S^T = [k, q] layout
    (exp on ScalarE with fused 1/sqrt(D) scale; an appended ones-column in v
    yields softmax denominators as an extra output row of the same matmul),
  - multiplies its y slice against its 128 rows of wp -> a full [T, C] partial.
Host sums the 8 partials and adds the output bias. Identical SPMD program on
all cores; per-core data arrives via the input maps.

Dtype strategy (flags below): bf16 runs the PE at 1 cycle/row and keeps the
HAM clock-gate warm; float32r is 2 cycles/row but ~13-bit accurate. The
softmax normalizer path always stays float32r/fp32.
"""

import numpy as np
import ml_dtypes

import concourse.bass as bass
import concourse.mybir as mybir
import concourse.tile as tile
from concourse import bacc
from concourse.bass_utils import run_bass_kernel_spmd

P = 128
N_EMBD = 1024
N_HEAD = 16
T_FULL = 4096
N_CORES = 8
D = N_EMBD // N_HEAD          # 64
C_LOCAL = 2 * D               # 128 channels per core (2 heads)
F32 = mybir.dt.float32
F32R = mybir.dt.float32r
BF16 = mybir.dt.bfloat16
ADD = mybir.AluOpType.add
MULT = mybir.AluOpType.mult
EXP = mybir.ActivationFunctionType.Exp
LOG = mybir.ActivationFunctionType.Ln if hasattr(mybir.ActivationFunctionType, "Ln") else mybir.ActivationFunctionType.Log
Y_LAG = 4  # y-matmul trails S/exp by this many k-blocks

QT = 512   # attention q-tile (matmul moving free dim)
TS = 512   # projection t-chunk

# dtype switches
PROJ_BF16 = True   # x / qkv-weight operands
ATT_BF16 = True     # qT/kT (S^T matmul) and P/v (y matmul) operands
OUT_BF16 = True    # yT / wp operands


def _np_dt(dt):
    return ml_dtypes.bfloat16 if dt == BF16 else np.float32


def build(T=T_FULL):
    n_ts = T // TS
    n_qt = T // QT
    n_kb = T // P
    n_ct = N_EMBD // P
    kb_per_qt = QT // P

    XW_DT = BF16 if PROJ_BF16 else F32R
    QK_DT = BF16 if ATT_BF16 else F32R
    PV_DT = BF16 if ATT_BF16 else F32R
    Y_DT = BF16 if OUT_BF16 else F32R

    nc = bacc.Bacc("TRN2", target_bir_lowering=False, debug=False,
                   num_devices=N_CORES)
    xdt = BF16 if PROJ_BF16 else F32
    xT = nc.dram_tensor("xT", [N_EMBD, T], xdt, kind="ExternalInput")
    # w3L is pre-swizzled on the host into the SBUF layout
    # [partition, c-tile, 3*C_LOCAL] so the DMA is fully contiguous.
    w3L = nc.dram_tensor("w3L", [P, (N_EMBD // P) * 3 * C_LOCAL], xdt,
                         kind="ExternalInput")
    b3 = nc.dram_tensor("b3", [C_LOCAL, 3], F32, kind="ExternalInput")
    wpdt = BF16 if OUT_BF16 else F32
    wpT = nc.dram_tensor("wpT", [C_LOCAL, N_EMBD], wpdt, kind="ExternalInput")
    tdt = BF16 if ATT_BF16 else F32
    tri = nc.dram_tensor("tri", [P, P], tdt, kind="ExternalInput")
    eye = nc.dram_tensor("eye", [P, P], tdt, kind="ExternalInput")
    ones = nc.dram_tensor("ones", [P, P], F32, kind="ExternalInput")
    sel2 = nc.dram_tensor("sel2", [P, P], F32, kind="ExternalInput")
    partial = nc.dram_tensor("partial", [T, N_EMBD], F32,
                             kind="ExternalOutput")

    def _in(ap, dt):
        return ap.bitcast(dt) if dt in (F32R,) else ap

    with tile.TileContext(nc) as tc:
        with (
            tc.tile_pool(name="const", bufs=1) as cpool,
            tc.tile_pool(name="psmall", bufs=2) as psmall,
            tc.tile_pool(name="pt", bufs=6) as ptp,
            tc.tile_pool(name="osb", bufs=2) as osbp,
        ):
            # ---- constants ----
            w3_sb = cpool.tile([P, n_ct, 3 * C_LOCAL], XW_DT, tag="w3")
            nc.sync.dma_start(
                w3_sb[:],
                _in(w3L.ap(), XW_DT).rearrange("p (co o) -> p co o", co=n_ct))
            wp_sb = cpool.tile([P, N_EMBD], Y_DT, tag="wp")
            nc.sync.dma_start(wp_sb[:], _in(wpT.ap(), Y_DT))
            b3_sb = cpool.tile([P, 3], F32, tag="b3")
            nc.sync.dma_start(b3_sb[:], b3.ap())
            tri_sb = cpool.tile([P, P], PV_DT, tag="tri")
            nc.sync.dma_start(tri_sb[:], _in(tri.ap(), PV_DT))
            eye_sb = cpool.tile([P, P], PV_DT, tag="eye")
            nc.sync.dma_start(eye_sb[:], _in(eye.ap(), PV_DT))
            ones_sb = cpool.tile([P, P], F32R, tag="ones")
            nc.sync.dma_start(ones_sb[:], ones.ap().bitcast(F32R))
            ones1_sb = ones_sb[0:1, :]
            sel2_sb = cpool.tile([P, P], F32R, tag="sel2")
            nc.sync.dma_start(sel2_sb[:], sel2.ap().bitcast(F32R))
            s2_sb = cpool.tile([P, QT], F32R, tag="s2p")
            nc.vector.tensor_copy(
                s2_sb[:], ones_sb[:, 0:1].to_broadcast((P, QT)))

            qT_sb = cpool.tile([P, T], QK_DT, tag="qT")
            kT_sb = cpool.tile([P, T], QK_DT, tag="kT")
            yT_sb = cpool.tile([P, T], Y_DT, tag="yT")
            # v in [t-part, (head0 d + one | head1 d + one)] layout
            v_sb = cpool.tile([P, n_kb, 2 * (D + 1)], PV_DT, tag="v")
            nc.vector.tensor_copy(
                v_sb[:, :, D], ones_sb[:, 0:1].to_broadcast((P, n_kb)))
            nc.vector.tensor_copy(
                v_sb[:, :, 2 * D + 1], ones_sb[:, 0:1].to_broadcast((P, n_kb)))

            # ---- q/k/v projections ----
            # x is fully resident in SBUF, fetched as whole rows (8 KiB
            # contiguous per partition) in two T-halves so the first
            # t-chunks' matmuls can start while the second half streams in.
            xfull = cpool.tile([P, n_ct, T], XW_DT, tag="xfull")
            T4 = T // 4
            for qtr in range(4):
                for c in range(n_ct):
                    nc.sync.dma_start(
                        xfull[:, c, qtr * T4:(qtr + 1) * T4],
                        _in(xT.ap(), XW_DT)[c * P:(c + 1) * P,
                                            qtr * T4:(qtr + 1) * T4])
            with (
                tc.tile_pool(name="xin", bufs=3) as xin,
                tc.tile_pool(name="proj_ps", bufs=2, space="PSUM") as pps,
                tc.tile_pool(name="tr_ps", bufs=2, space="PSUM") as trps,
            ):
                XCH = min(2 * TS, T)
                NHF = XCH // TS
                for tsp in range(T // XCH):
                    prj = [[pps.tile([P, TS], F32, tag=f"prj{j}",
                                     name=f"prj{j}_{tsp}_{hf}")
                            for j in range(3)] for hf in range(NHF)]
                    for c in range(n_ct):
                        for hf in range(NHF):
                            for j in range(3):
                                nc.tensor.matmul(
                                    prj[hf][j][:],
                                    w3_sb[:, c, j * P:(j + 1) * P],
                                    xfull[:, c, tsp * XCH + hf * TS:
                                          tsp * XCH + (hf + 1) * TS],
                                    start=(c == 0), stop=(c == n_ct - 1))
                    for hf in range(NHF):
                        t0 = tsp * XCH + hf * TS
                        # bias add + copy out of PSUM
                        nc.vector.tensor_tensor(
                            qT_sb[:, t0:t0 + TS], prj[hf][0][:],
                            b3_sb[:, 0:1].to_broadcast((P, TS)), ADD)
                        nc.vector.tensor_tensor(
                            kT_sb[:, t0:t0 + TS], prj[hf][1][:],
                            b3_sb[:, 1:2].to_broadcast((P, TS)), ADD)
                        vt_t = xin.tile([P, TS], PV_DT, tag="vt")
                        nc.vector.tensor_tensor(
                            vt_t[:], prj[hf][2][:],
                            b3_sb[:, 2:3].to_broadcast((P, TS)), ADD)
                        # transpose v chunk into [t, c_local] layout
                        for kk in range(TS // P):
                            gkb = (t0 + kk * P) // P
                            t_ps = trps.tile([P, P], PV_DT, tag="tr")
                            nc.tensor.transpose(
                                t_ps[:], vt_t[:, kk * P:(kk + 1) * P],
                                eye_sb[:])
                            for h in range(2):
                                nc.vector.tensor_copy(
                                    v_sb[:, gkb, h * (D + 1):h * (D + 1) + D],
                                    t_ps[:, h * D:(h + 1) * D])

            # ---- attention + interleaved output projection ----
            with (
                tc.tile_pool(name="st_ps", bufs=2, space="PSUM") as stps,
                tc.tile_pool(name="y_ps", bufs=1, space="PSUM") as yps,
                tc.tile_pool(name="o_ps", bufs=1, space="PSUM") as ops,
            ):
                def emit_outproj(qt):
                    # output projection for the 4 t-blocks of q-tile qt
                    for tb in range(qt * (QT // P), (qt + 1) * (QT // P)):
                        o_ps = [ops.tile([P, 512], F32, tag=f"o{i}",
                                         name=f"o{i}_{tb}") for i in range(2)]
                        o_sb = osbp.tile([P, N_EMBD], F32, tag="osb")
                        for i in range(2):
                            nc.tensor.matmul(
                                o_ps[i][:],
                                yT_sb[:, tb * P:(tb + 1) * P],
                                wp_sb[:, i * 512:(i + 1) * 512],
                                start=True, stop=True)
                            nc.vector.tensor_copy(
                                o_sb[:, i * 512:(i + 1) * 512], o_ps[i][:])
                        nc.sync.dma_start(
                            partial.ap()[tb * P:(tb + 1) * P, :], o_sb[:])

                from collections import deque
                pend_y = deque()
                pending_norm = None
                pending_out = None
                pending_tail = None

                def finish_norm(nqt, y_ps_n):
                    R_ps = ops.tile([P, QT], F32, tag="o0", name=f"R_{nqt}")
                    nc.tensor.matmul(R_ps[:], sel2_sb[:], s2_sb[:],
                                     start=True, stop=True)
                    R_sb = psmall.tile([P, QT], F32, tag="Rs")
                    nc.vector.reciprocal_approx_fast(R_sb[:], R_ps[:])
                    for h in range(2):
                        nc.vector.tensor_tensor(
                            yT_sb[D * h:D * (h + 1),
                                  nqt * QT:(nqt + 1) * QT],
                            y_ps_n[h][0:D, :],
                            R_sb[D * h:D * (h + 1), :],
                            MULT)

                for qt in range(n_qt):
                    q0 = qt * QT
                    nkb = (qt + 1) * kb_per_qt
                    y_ps = [yps.tile([P, QT], F32, tag=f"y{h}",
                                     name=f"y{h}_{qt}") for h in range(2)]

                    # software-pipelined: y-matmul trails S/exp by Y_LAG;
                    # the tail of each q-tile (last y-matmuls + sum copies)
                    # is deferred into the next q-tile's stream.
                    def emit_y(kb, p_pair, y_ps_l, nkb_l, q_first):
                        dg = kb >= q_first
                        cc = (kb - q_first) * P if dg else 0
                        for h in range(2):
                            nc.tensor.matmul(
                                y_ps_l[h][0:D + 1, cc:QT],
                                v_sb[:, kb, h * (D + 1):(h + 1) * (D + 1)],
                                p_pair[:, h * QT + cc:(h + 1) * QT],
                                start=(kb == 0), stop=(kb == nkb_l - 1))

                    for kb in range(nkb):
                        diag = kb >= qt * kb_per_qt
                        c = (kb - qt * kb_per_qt) * P if diag else 0
                        # both heads' scores in one 2-bank tile -> one exp
                        s_pair = stps.tile([P, 2 * QT], F32, tag="s",
                                           name=f"s_{qt}_{kb}")
                        for h in range(2):
                            nc.tensor.matmul(
                                s_pair[:, h * QT + c:(h + 1) * QT],
                                kT_sb[D * h:D * h + D, kb * P:(kb + 1) * P],
                                qT_sb[D * h:D * h + D, q0 + c:q0 + QT],
                                start=True, stop=True)
                        p_pair = ptp.tile([P, 2 * QT], PV_DT, tag="p",
                                          name=f"p_{qt}_{kb}")
                        if diag:
                            nc.scalar.activation(
                                p_pair[:].rearrange(
                                    "p (h q) -> p h q", h=2)[:, :, c:QT],
                                s_pair[:].rearrange(
                                    "p (h q) -> p h q", h=2)[:, :, c:QT],
                                EXP, scale=1.0 / float(np.sqrt(D)))
                            for h in range(2):
                                nc.vector.tensor_tensor(
                                    p_pair[:, h * QT + c:h * QT + c + P],
                                    p_pair[:, h * QT + c:h * QT + c + P],
                                    tri_sb[:], MULT)
                        else:
                            nc.scalar.activation(
                                p_pair[:], s_pair[:], EXP,
                                scale=1.0 / float(np.sqrt(D)))
                        if kb == 0 and pending_tail is not None:
                            pending_tail()
                            pending_tail = None
                        if kb == 1 and pending_norm is not None:
                            finish_norm(*pending_norm)
                            pending_norm = None
                        if kb == 2 and pending_out is not None:
                            emit_outproj(pending_out)
                            pending_out = None
                        pend_y.append(
                            (lambda a, b, c_, d_, e_:
                             lambda: emit_y(a, b, c_, d_, e_))(
                                 kb, p_pair, y_ps, nkb, qt * kb_per_qt))
                        while len(pend_y) > Y_LAG:
                            pend_y.popleft()()

                    def make_tail(y_ps_t, ndrain):
                        def _tail():
                            for _ in range(ndrain):
                                pend_y.popleft()()
                            for h in range(2):
                                nc.vector.tensor_copy(
                                    s2_sb[64 * h:64 * h + 1, :],
                                    y_ps_t[h][D:D + 1, :])
                        return _tail

                    pending_tail = make_tail(y_ps, len(pend_y))
                    if pending_out is not None:
                        emit_outproj(pending_out)
                        pending_out = None
                    pending_norm = (qt, y_ps)
                    pending_out = qt
                if pending_tail is not None:
                    pending_tail()
                if pending_norm is not None:
                    finish_norm(*pending_norm)
                emit_outproj(pending_out)

    nc.compile()
    return nc


def make_core_inputs(x, wq, bq, wk, bk, wv, bv, wp):
    """Per-core input dicts (host-side sharding). x: [T, C] fp32."""
    xw_np = _np_dt(BF16 if PROJ_BF16 else F32)
    y_np = _np_dt(BF16 if OUT_BF16 else F32)
    t_np = _np_dt(BF16 if ATT_BF16 else F32)
    xT = np.ascontiguousarray(x.T).astype(xw_np)
    tri = (np.arange(P)[None, :] >= np.arange(P)[:, None]).astype(t_np)
    eye = np.eye(P, dtype=np.float32).astype(t_np)
    ones = np.ones((P, P), dtype=np.float32)
    sel2 = np.zeros((P, P), dtype=np.float32)
    sel2[0, :D] = 1.0
    sel2[64, D:2 * D] = 1.0
    in_maps = []
    for core in range(N_CORES):
        ch = slice(core * C_LOCAL, (core + 1) * C_LOCAL)
        w3T = np.concatenate([wq[ch].T, wk[ch].T, wv[ch].T], axis=1)
        w3L = np.ascontiguousarray(
            w3T.reshape(N_EMBD // P, P, 3 * C_LOCAL).transpose(1, 0, 2)
            .reshape(P, -1)).astype(xw_np)
        b3 = np.ascontiguousarray(
            np.stack([bq[ch], bk[ch], bv[ch]], axis=1)).astype(np.float32)
        wpT = np.ascontiguousarray(wp[:, ch].T).astype(y_np)
        in_maps.append({
            "xT": xT, "w3L": w3L, "b3": b3, "wpT": wpT,
            "tri": tri, "eye": eye, "ones": ones, "sel2": sel2,
        })
    return in_maps


_NC_CACHE = {}


def _get_nc(T):
    if T not in _NC_CACHE:
        _NC_CACHE[T] = build(T)
    return _NC_CACHE[T]


def kernel(x, wq, bq, wk, bk, wv, bv, wp, bp, trace=False):
    x = np.asarray(x)
    B, T, C = x.shape
    in_maps = make_core_inputs(
        x[0], np.asarray(wq), np.asarray(bq), np.asarray(wk), np.asarray(bk),
        np.asarray(wv), np.asarray(bv), np.asarray(wp))
    nc = _get_nc(T)
    res = run_bass_kernel_spmd(nc, in_maps, core_ids=list(range(N_CORES)),
                               trace=trace)
    out = res.results[0]["partial"].copy()
    for c in range(1, N_CORES):
        out += res.results[c]["partial"]
    out += np.asarray(bp)[None, :]
    kernel.last_result = res
    return out[None].astype(np.float32)
